# revision 83
# baseline (speedup 1.0000x reference)
"""Trainium2 Bass kernel for nn_BGCH (2-layer GNN message passing + binary hash).

Computation (see reference):
  u = random_projection(user_emb, v0_u); i = random_projection(item_emb, v0_i)
  x0 = concat(u, i)                                  [90000, 256]
  H0 = sign(x0 @ W.T)
  x1 = spmm(A, x0); H1 = sign(x1 @ W.T)
  x2 = spmm(A, x1); H2 = sign(x2 @ W.T)
  out = concat([H0, H1, H2], axis=1)                 [90000, 768]

Distribution: nodes sharded over 8 cores (89 row-blocks of 128 per core),
edges partitioned by destination row. Per layer each core gathers x[col]
rows (fp16, 512B) from a replicated DRAM table via gpsimd.dma_gather
(int16 indices, 4 SWDGE queues in parallel), does a segmented-sum via
TensorE matmul with a host-precomputed fp16 S^T scatter matrix (one val
per edge slot at its dest-row column), then the dense hash locally in
fp32. The replicated table is rebuilt between layers by 3 piece-wise
AllGathers (pieces of 32 k-blocks = exactly one 32768-row gather chunk
in a piece-major slot layout), so collectives overlap compute and
chunk-c gathers start as soon as piece c lands. The random projection
is applied as a rank-1 update X' = X - c (Xv) v^T on DVE (v from a
Gram-matrix power iteration, Gram AllReduced across cores); the hash
weight is pre-projected the same way (W0 = W^T - (cv)(Wv)^T).
Numerics: fp16 table+vals cost ~4.8K sign flips of the ~6.9K budget
(rel_err 2e-2); hash stays fp32. S_HILO=True falls back to fp16 hi/lo
scatter vals (2 matmuls/tile) if more margin is needed.
"""
import sys
sys.path.insert(0, "/opt/trn_rl_repo")

import numpy as np


# ---------------------------------------------------------------- config
class Cfg:
    N_USERS = 30000
    N_ITEMS = 60000
    CON_DIM = 256
    N_LAYERS = 2
    RP_ITER = 2
    RP_ETA = 0.5
    RP_AVG = 3
    N_CORES = 8
    U_BLOCKS = 240          # user block slots (multiple of N_CORES)
    I_BLOCKS = 472          # item block slots (multiple of N_CORES)
    CHUNK = 32768           # int16 gather-index range
    PIECE_K = 32            # k-blocks per AllGather piece (32*8*128 = CHUNK)
    GMERGE = 1              # blocks per merged gather group (divides PIECE_K)
    PREFETCH_W = 8          # layer-start chunk-0/1 gather warm-up depth
    BAL_ITERS = 60000       # k-slot swap polish iterations (pad reduction)
    S_HILO = False          # True: fp16 hi/lo scatter vals (2 matmuls/tile)
    S_ONFLY = True          # build S^T tiles on DVE ((iota==rloc)*val)
    DEBUG_X = False

    @property
    def SCOLS(self):        # S^T columns per tile
        return 256 if self.S_HILO else 128

    @property
    def BLOCKS(self):
        return self.U_BLOCKS + self.I_BLOCKS

    @property
    def KPC(self):  # blocks per core
        return self.BLOCKS // self.N_CORES

    @property
    def U_KPC(self):
        return self.U_BLOCKS // self.N_CORES

    @property
    def N_SLOTS(self):
        return self.BLOCKS * 128

    @property
    def ROWS_PC(self):
        return self.KPC * 128

    @property
    def N_CHUNKS(self):
        return (self.N_SLOTS + self.CHUNK - 1) // self.CHUNK

    @property
    def PIECES(self):       # [(k0, k1)] per piece; pieces tile the CHUNKs
        out = []
        k = 0
        while k < self.KPC:
            out.append((k, min(k + self.PIECE_K, self.KPC)))
            k += self.PIECE_K
        return out


# ------------------------------------------------------------- host prep
def _pack_nodes(deg, nblocks, b0, rng):
    """Greedy bin-pack nodes into blocks of <=128 rows, balancing edge load.
    Returns (blk, rowin) arrays."""
    import heapq
    n = len(deg)
    order = np.argsort(-deg, kind="stable")
    heap = [(0, b0 + i, 0) for i in range(nblocks)]
    heapq.heapify(heap)
    blk = np.empty(n, np.int32)
    rowin = np.empty(n, np.int32)
    for node in order:
        while True:
            load, b, cnt = heapq.heappop(heap)
            if cnt < 128:
                break
            # block full: drop it from the heap permanently
        blk[node] = b
        rowin[node] = cnt
        heapq.heappush(heap, (load + int(deg[node]), b, cnt + 1))
    return blk, rowin


def host_prep(cfg, user_emb, item_emb, edge_val, edge_row, edge_col):
    """Node->slot permutation (piece-major table layout), block->core
    assignment, per-core edge tile template (uniform across cores) and
    packed device input arrays."""
    N = cfg.N_USERS + cfg.N_ITEMS
    rng = np.random.default_rng(0)
    deg = np.bincount(edge_row, minlength=N)

    ub, ur = _pack_nodes(deg[:cfg.N_USERS], cfg.U_BLOCKS, 0, rng)
    ib, ir = _pack_nodes(deg[cfg.N_USERS:], cfg.I_BLOCKS, cfg.U_BLOCKS, rng)
    blk = np.concatenate([ub, ib])
    rowin = np.concatenate([ur, ir])

    # block -> (core, k); user blocks get k in [0, U_KPC), items [U_KPC, KPC)
    bload = np.bincount(blk[edge_row], minlength=cfg.BLOCKS)
    core_of = np.empty(cfg.BLOCKS, np.int32)
    k_of = np.empty(cfg.BLOCKS, np.int32)
    for lo, hi, k0 in ((0, cfg.U_BLOCKS, 0), (cfg.U_BLOCKS, cfg.BLOCKS, cfg.U_KPC)):
        ids = np.arange(lo, hi)
        order = ids[np.argsort(-bload[lo:hi], kind="stable")]
        cnt = np.zeros(cfg.N_CORES, np.int32)
        for i, b in enumerate(order):
            rnd, pos = divmod(i, cfg.N_CORES)
            c = pos if rnd % 2 == 0 else cfg.N_CORES - 1 - pos
            core_of[b] = c
            k_of[b] = k0 + cnt[c]
            cnt[c] += 1

    # piece-major global slot: piece p holds all cores' k in [k0p, k1p),
    # core-major within the piece, so AllGather piece output == table piece
    pieces = cfg.PIECES
    pk0 = np.empty(cfg.KPC, np.int64)     # piece start k, per k
    pkn = np.empty(cfg.KPC, np.int64)     # piece size in blocks, per k
    pbase = np.empty(cfg.KPC, np.int64)   # piece base slot, per k
    base = 0
    for (k0, k1) in pieces:
        pk0[k0:k1] = k0
        pkn[k0:k1] = k1 - k0
        pbase[k0:k1] = base
        base += cfg.N_CORES * (k1 - k0) * 128

    def slots_of_all_nodes():
        c_, k_ = core_of[blk], k_of[blk]
        return pbase[k_] + (c_ * pkn[k_] + (k_ - pk0[k_])) * 128 + rowin

    # within each core+side, order k slots by chunk-0 edge count to reduce
    # cross-core template padding
    slot_of_node = slots_of_all_nodes()
    chunk_of_node = slot_of_node // cfg.CHUNK
    ccount = np.zeros((cfg.N_CORES, cfg.KPC, cfg.N_CHUNKS), np.int64)
    np.add.at(ccount, (core_of[blk[edge_row]], k_of[blk[edge_row]],
                       chunk_of_node[edge_col]), 1)
    for c in range(cfg.N_CORES):
        for lo, hi in ((0, cfg.U_KPC), (cfg.U_KPC, cfg.KPC)):
            seg = ccount[c, lo:hi]
            key = seg[:, 0] * 1000000 + seg[:, 1]
            perm = np.argsort(key, kind="stable")
            mask = (core_of == c) & (k_of >= lo) & (k_of < hi)
            bids = np.where(mask)[0]
            old_k = k_of[bids] - lo
            inv = np.empty_like(perm)
            inv[perm] = np.arange(len(perm))
            k_of[bids] = lo + inv[old_k]
            ccount[c, lo:hi] = seg[perm]

    slot_of_node = slots_of_all_nodes()
    chunk_of_node = slot_of_node // cfg.CHUNK
    local_of_node = core_of[blk] * cfg.ROWS_PC + k_of[blk] * 128 + rowin

    # recompute actual per-(core,k,chunk) counts after the k reorder
    ccount = np.zeros((cfg.N_CORES, cfg.KPC, cfg.N_CHUNKS), np.int64)
    np.add.at(ccount, (core_of[blk[edge_row]], k_of[blk[edge_row]],
                       chunk_of_node[edge_col]), 1)

    # local-search polish: swap k slots within (core, side) to shrink the
    # padded template sum_{k,c} ceil(max_core/128). NOTE: moving a block to
    # a different k changes its piece, hence its chunk for SOURCE purposes,
    # so only swap within the same piece (keeps chunk_of_node valid).
    bids = np.full((cfg.N_CORES, cfg.KPC), -1, np.int64)
    bids[core_of, k_of] = np.arange(cfg.BLOCKS)
    cc = ccount
    rng2 = np.random.default_rng(1)

    def col_tiles(k):
        return int(np.ceil(cc[:, k, :].max(axis=0) / 128.0).sum())

    # swap candidates: same core, same side, same piece
    cand = []
    for (pk0_, pk1_) in pieces:
        for lo, hi in ((0, cfg.U_KPC), (cfg.U_KPC, cfg.KPC)):
            a, b = max(lo, pk0_), min(hi, pk1_)
            if b - a >= 2:
                cand.append((a, b))
    for it in range(cfg.BAL_ITERS):
        c = it % cfg.N_CORES
        a, b = cand[rng2.integers(len(cand))]
        j1, j2 = rng2.integers(a, b, 2)
        if j1 == j2:
            continue
        before = col_tiles(j1) + col_tiles(j2)
        cc[c, [j1, j2], :] = cc[c, [j2, j1], :]
        if col_tiles(j1) + col_tiles(j2) > before:
            cc[c, [j1, j2], :] = cc[c, [j2, j1], :]
        else:
            b1, b2 = bids[c, j1], bids[c, j2]
            bids[c, j1], bids[c, j2] = b2, b1
            k_of[b1], k_of[b2] = j2, j1

    slot_of_node = slots_of_all_nodes()
    chunk_of_node = slot_of_node // cfg.CHUNK
    local_of_node = core_of[blk] * cfg.ROWS_PC + k_of[blk] * 128 + rowin
    ccount = np.zeros((cfg.N_CORES, cfg.KPC, cfg.N_CHUNKS), np.int64)
    np.add.at(ccount, (core_of[blk[edge_row]], k_of[blk[edge_row]],
                       chunk_of_node[edge_col]), 1)

    # template: tiles per (k, chunk) = ceil(max over cores / 128)
    T = np.ceil(ccount.max(axis=0) / 128).astype(np.int64)  # [KPC, N_CHUNKS]
    tiles_per_block = T.sum(axis=1)                          # [KPC]
    tot_tiles = int(tiles_per_block.sum())
    tot_slots = tot_tiles * 128

    # per-edge fields
    e_blk = blk[edge_row]
    e_core = core_of[e_blk]
    e_k = k_of[e_blk]
    e_rloc = rowin[edge_row]
    e_src = slot_of_node[edge_col]
    e_chunk = e_src // cfg.CHUNK
    e_idx = (e_src % cfg.CHUNK).astype(np.int16)

    # stream order (group, chunk, k): gathers merge the G blocks of a group
    # into one dma_gather per (group, chunk)
    groups = []
    for (k0, k1) in pieces:
        k = k0
        while k < k1:
            groups.append(list(range(k, min(k + cfg.GMERGE, k1))))
            k += cfg.GMERGE
    seg_rank = np.zeros((cfg.KPC, cfg.N_CHUNKS), np.int64)
    seg_tile_off = np.zeros((cfg.KPC, cfg.N_CHUNKS), np.int64)
    acc = 0
    rank = 0
    for grp_ks in groups:
        for c in range(cfg.N_CHUNKS):
            for k in grp_ks:
                seg_rank[k, c] = rank
                rank += 1
                seg_tile_off[k, c] = acc
                acc += T[k, c]

    idx_all = np.zeros((cfg.N_CORES, tot_slots), np.int16)
    rloc_all = np.zeros((cfg.N_CORES, tot_slots), np.float32)
    val_all = np.zeros((cfg.N_CORES, tot_slots), np.float32)
    e_rank = seg_rank[e_k, e_chunk]
    order = np.lexsort((e_rank, e_core))
    eo_core = e_core[order]
    eo_rank = e_rank[order]
    grp = eo_core.astype(np.int64) * (cfg.KPC * cfg.N_CHUNKS) + eo_rank
    uniq, first = np.unique(grp, return_index=True)
    within = np.arange(len(grp)) - np.repeat(first, np.diff(np.append(first, len(grp))))
    pos = seg_tile_off[e_k[order], e_chunk[order]] * 128 + within
    idx_all[eo_core, pos] = e_idx[order]
    rloc_all[eo_core, pos] = e_rloc[order].astype(np.float32)
    val_all[eo_core, pos] = edge_val[order]

    # device layouts
    idx_lay = np.zeros((cfg.N_CORES, 128, tot_slots // 16), np.int16)
    wrap = idx_all.reshape(cfg.N_CORES, -1, 16)
    for rep in range(8):
        idx_lay[:, rep * 16:(rep + 1) * 16, :] = wrap.transpose(0, 2, 1)

    # S^T tiles: one nonzero per partition (edge slot) at column rloc
    # (dest row within block).
    if cfg.S_ONFLY:
        # device builds S = (iota == rloc) * val on DVE; ship only the
        # per-(partition, tile) rloc and val planes (fp16)
        rv_lay = np.stack([
            rloc_all.reshape(cfg.N_CORES, tot_tiles, 128).transpose(0, 2, 1),
            val_all.reshape(cfg.N_CORES, tot_tiles, 128).transpose(0, 2, 1),
        ], axis=2)  # [cores, 128, 2, tot_tiles]
        st_lay = np.ascontiguousarray(
            rv_lay.reshape(cfg.N_CORES, 128, 2 * tot_tiles).astype(np.float16))
    else:
        SC = cfg.SCOLS
        v_hi = val_all.astype(np.float16)
        st_lay = np.zeros((cfg.N_CORES, 128, tot_tiles * SC), np.float16)
        cidx = np.repeat(np.arange(cfg.N_CORES), tot_slots)
        pp_ = np.tile(np.arange(tot_slots) % 128, cfg.N_CORES)
        tt_ = np.tile(np.arange(tot_slots) // 128, cfg.N_CORES)
        rl_ = rloc_all.reshape(-1).astype(np.int64)
        st_lay[cidx, pp_, tt_ * SC + rl_] = v_hi.reshape(-1)
        if cfg.S_HILO:
            v_lo = (val_all - v_hi.astype(np.float32)).astype(np.float16)
            st_lay[cidx, pp_, tt_ * SC + 128 + rl_] = v_lo.reshape(-1)

    # packed embeddings per core (local k-major layout)
    emb_all = np.zeros((cfg.N_CORES * cfg.ROWS_PC, cfg.CON_DIM), np.float32)
    emb_all[local_of_node[:cfg.N_USERS]] = user_emb
    emb_all[local_of_node[cfg.N_USERS:]] = item_emb
    emb_pc = emb_all.reshape(cfg.N_CORES, cfg.ROWS_PC, cfg.CON_DIM)

    return dict(
        local_of_node=local_of_node, T=T, tiles_per_block=tiles_per_block,
        seg_tile_off=seg_tile_off, tot_tiles=tot_tiles, tot_slots=tot_slots,
        idx_lay=idx_lay, st_lay=st_lay, emb_pc=emb_pc, groups=groups,
    )


# ------------------------------------------------------------ bass build
def build_program(cfg, prep):
    import concourse.bacc as bacc
    import concourse.mybir as mybir
    import concourse.tile as tile

    dt = mybir.dt
    F = cfg.CON_DIM
    KPC = cfg.KPC
    T = prep["T"]
    seg_off = prep["seg_tile_off"]
    tiles_pb = prep["tiles_per_block"]
    tot_tiles = prep["tot_tiles"]
    tot_slots = prep["tot_slots"]
    NCH = cfg.N_CHUNKS
    SC = cfg.SCOLS
    pieces = cfg.PIECES
    groups = prep["groups"]
    AFT = mybir.ActivationFunctionType
    ALU = mybir.AluOpType

    nc = bacc.Bacc("TRN2", target_bir_lowering=False, debug=False,
                   num_devices=cfg.N_CORES, num_swdge_queues=4)

    emb_d = nc.dram_tensor("emb", [cfg.ROWS_PC, F], dt.float32, kind="ExternalInput")
    idx_d = nc.dram_tensor("idx", [128, tot_slots // 16], dt.int16, kind="ExternalInput")
    MAXT = max(int(sum(tiles_pb[k] for k in grp_ks)) for grp_ks in groups)
    if cfg.S_ONFLY:
        st_d = nc.dram_tensor("st", [128, 2 * tot_tiles], dt.float16,
                              kind="ExternalInput")
        iotab_d = nc.dram_tensor("iotab", [128, MAXT * 128], dt.float16,
                                 kind="ExternalInput")
    else:
        st_d = nc.dram_tensor("st", [128, tot_tiles * SC], dt.float16,
                              kind="ExternalInput")
    wt_d = nc.dram_tensor("wt", [F, F], dt.float32, kind="ExternalInput")    # W^T
    v0_d = nc.dram_tensor("v0", [128, 2 * 2 * cfg.RP_AVG], dt.float32, kind="ExternalInput")
    ident_d = nc.dram_tensor("ident", [128, 128], dt.float32, kind="ExternalInput")
    ones_d = nc.dram_tensor("ones1", [1, 128], dt.float32, kind="ExternalInput")
    out_d = nc.dram_tensor("out", [cfg.ROWS_PC, 3 * F], dt.int8, kind="ExternalOutput")
    xdbg = None
    if cfg.DEBUG_X:
        xdbg = [nc.dram_tensor(f"xdbg{i}", [cfg.ROWS_PC, F], dt.float32,
                               kind="ExternalOutput") for i in range(3)]

    # internal DRAM: fp16 x tables, piece-major. One tab tensor per gather
    # CHUNK; each AllGather piece writes its row-slice of its chunk's tab.
    piece_rows = [cfg.N_CORES * (k1 - k0) * 128 for (k0, k1) in pieces]
    piece_base = np.concatenate([[0], np.cumsum(piece_rows)])

    def piece_tensors(nm):
        ag, tab = [], []
        for p, (k0, k1) in enumerate(pieces):
            ag.append(nc.dram_tensor(f"{nm}_ag{p}", [(k1 - k0) * 128, F],
                                     dt.float16))
        for c in range(NCH):
            rows = min(cfg.N_SLOTS, (c + 1) * cfg.CHUNK) - c * cfg.CHUNK
            tab.append(nc.dram_tensor(f"{nm}_tab{c}", [rows, F], dt.float16,
                                      addr_space="Shared"))
        return ag, tab

    ag0, tab0 = piece_tensors("t0")
    ag1, tab1 = piece_tensors("t1")
    gr_in = nc.dram_tensor("gr_in", [4 * 128, F], dt.float32)
    gr_out = nc.dram_tensor("gr_out", [4 * 128, F], dt.float32)

    rg = [list(range(cfg.N_CORES))]
    piece_of = {}
    for q, (k0q, k1q) in enumerate(pieces):
        for kk in range(k0q, k1q):
            piece_of[kk] = q
    # piece p's inputs complete at block k1-1; fire 3 blocks later (capped)
    fire_at = {min(k1 - 1 + 3, cfg.KPC - 1): q
               for q, (k0, k1) in enumerate(pieces)}

    def fire_piece(ag, tab, p):
        r0 = int(piece_base[p])
        c = r0 // cfg.CHUNK
        off = r0 - c * cfg.CHUNK
        nc.gpsimd.collective_compute(
            "AllGather", ALU.bypass, replica_groups=rg,
            ins=[ag[p].ap().opt()],
            outs=[tab[c][off:off + piece_rows[p], :].opt()])

    with tile.TileContext(nc) as tc:
        with tc.tile_pool(name="const", bufs=1) as cpool:
            # preload the gpsimd library holding DMAGatherAnt so the ~60us
            # reload overlaps the Gram phase instead of stalling layer 1
            from concourse import library_config
            nc.gpsimd.load_library(library_config.mlp)
            ident_sb = cpool.tile([128, 128], dt.float32, tag="ident")
            nc.sync.dma_start(ident_sb[:], ident_d[:])
            ones_sb = cpool.tile([1, 128], dt.float32, tag="ones1")
            nc.sync.dma_start(ones_sb[:], ones_d[:])
            wt_sb = cpool.tile([128, 2, F], dt.float32, tag="wt")
            nc.sync.dma_start(wt_sb[:, 0, :], wt_d[0:128, :])
            nc.sync.dma_start(wt_sb[:, 1, :], wt_d[128:256, :])
            v0_sb = cpool.tile([128, 2, 2, cfg.RP_AVG], dt.float32, tag="v0")
            nc.sync.dma_start(v0_sb[:], v0_d[:])
            idx_sb = cpool.tile([128, tot_slots // 16], dt.int16, tag="idx")
            nc.sync.dma_start(idx_sb[:], idx_d[:])
            if cfg.S_ONFLY:
                rv_sb = cpool.tile([128, 2, tot_tiles], dt.float16, tag="rv")
                nc.sync.dma_start(rv_sb[:], st_d[:])
                iotab_sb = cpool.tile([128, MAXT, 128], dt.float16, tag="iotab")
                nc.sync.dma_start(iotab_sb[:], iotab_d[:])

            # =================== RP phase a: Gram matrices ===================
            # emb is DMAed once into a persistent stash; phase d reads it
            # from SBUF (stash pool closed before the layers)
            stash_cm = tc.tile_pool(name="stash", bufs=1)
            stpool = stash_cm.__enter__()
            xst = stpool.tile([128, KPC, F], dt.float32, tag="xst")
            with tc.tile_pool(name="rpa_ps", bufs=1, space="PSUM") as gpsum:
                pg = [gpsum.tile([128, F], dt.float32, tag=f"g{i}", name=f"pg{i}") for i in range(4)]
                # pg[0..1] = Gram_u chunks, pg[2..3] = Gram_i chunks
                # symmetric Gram: row-chunk 0 computes [G00|G01] fully; for
                # chunk 1 only G11 (G10 = G01^T, rebuilt after the AllReduce)
                for k in range(KPC):
                    nc.sync.dma_start(xst[:, k, :], emb_d[k * 128:(k + 1) * 128, :])
                    side = 0 if k < cfg.U_KPC else 1
                    first = k == 0 or k == cfg.U_KPC
                    last = k == cfg.U_KPC - 1 or k == KPC - 1
                    nc.tensor.matmul(pg[2 * side][:],
                                     xst[:, k, 0:128],
                                     xst[:, k, :], start=first, stop=last)
                    nc.tensor.matmul(pg[2 * side + 1][:, 128:256],
                                     xst[:, k, 128:256],
                                     xst[:, k, 128:256], start=first, stop=last)
                g_sb = cpool.tile([128, 4, F], dt.float32, tag="gsb")
                for i in range(4):
                    if i % 2 == 0:
                        nc.scalar.activation(g_sb[:, i, :], pg[i][:], AFT.Copy)
                    else:
                        nc.scalar.activation(g_sb[:, i, 128:256],
                                             pg[i][:, 128:256], AFT.Copy)
                        nc.vector.memset(g_sb[:, i, 0:128], 0)
                for i in range(4):
                    nc.sync.dma_start(gr_in[i * 128:(i + 1) * 128, :], g_sb[:, i, :])
            nc.gpsimd.collective_compute(
                "AllReduce", ALU.add, replica_groups=rg,
                ins=[gr_in.ap().opt()], outs=[gr_out.ap().opt()])

            # ====== RP phase b/c: v, coef, W0 = W^T - (c v)(W v)^T per side ======
            vrow2 = cpool.tile([1, 2, F], dt.float32, tag="vrow2")      # v^T per side
            vb2 = cpool.tile([128, 2, F], dt.float32, tag="vb2")        # bcast v^T
            ncoef2 = cpool.tile([128, 2, 1], dt.float32, tag="ncoef2")  # -eta/vTv
            w0_sb = cpool.tile([128, 2, 2, F], dt.float32, tag="w0sb")
            v2_sb = cpool.tile([128, 2, 2, 1], dt.float32, tag="v2")    # [.,side,jc,.]
            with tc.tile_pool(name="rpb", bufs=1) as vpool, \
                 tc.tile_pool(name="rpb_ps", bufs=1, space="PSUM") as vpsum:
                gg = vpool.tile([128, 4, F], dt.float32, tag="gg")
                for i in range(4):
                    nc.sync.dma_start(gg[:, i, :], gr_out[i * 128:(i + 1) * 128, :])
                # rebuild G10 = G01^T in the row-chunk-1 slots
                for side in range(2):
                    ptg = vpsum.tile([128, 128], dt.float32, tag="pvb")
                    nc.tensor.transpose(ptg[:], gg[:, 2 * side, 128:256],
                                        ident_sb[:])
                    nc.scalar.activation(gg[:, 2 * side + 1, 0:128], ptg[:],
                                         AFT.Copy)
                for side in range(2):
                    cur = None
                    for it in range(cfg.RP_ITER):
                        pv = [vpsum.tile([128, cfg.RP_AVG], dt.float32, tag=f"pv{ic}",
                                        name=f"pv{ic}") for ic in range(2)]
                        for ic in range(2):
                            for kc in range(2):
                                rhs_ap = (v0_sb[:, side, kc, :] if cur is None
                                          else cur[:, kc, :])
                                nc.tensor.matmul(
                                    pv[ic][:],
                                    gg[:, 2 * side + kc, ic * 128:(ic + 1) * 128],
                                    rhs_ap,
                                    start=(kc == 0), stop=(kc == 1))
                        nxt = vpool.tile([128, 2, cfg.RP_AVG], dt.float32, tag=f"vk{it}")
                        for ic in range(2):
                            nc.scalar.activation(nxt[:, ic, :], pv[ic][:], AFT.Copy)
                        cur = nxt
                    # v = mean over restarts
                    for ic in range(2):
                        vsum = vpool.tile([128, 1], dt.float32, tag="vs")
                        nc.vector.tensor_reduce(vsum[:], cur[:, ic, :],
                                                mybir.AxisListType.X, ALU.add)
                        nc.scalar.activation(v2_sb[:, side, ic, :], vsum[:], AFT.Copy,
                                             scale=1.0 / cfg.RP_AVG)
                    # vTv
                    pn = vpsum.tile([1, 1], dt.float32, tag="pn")
                    for ic in range(2):
                        nc.tensor.matmul(pn[:], v2_sb[:, side, ic, :], v2_sb[:, side, ic, :],
                                         start=(ic == 0), stop=(ic == 1))
                    recip = vpool.tile([1, 1], dt.float32, tag="rec")
                    nc.vector.reciprocal(recip[:], pn[:])
                    # broadcast -eta/vTv to all partitions
                    pb = vpsum.tile([128, 1], dt.float32, tag="pb")
                    nc.tensor.matmul(pb[:], ones_sb[:], recip[:], start=True, stop=True)
                    nc.scalar.activation(ncoef2[:, side, :], pb[:], AFT.Copy,
                                         scale=-cfg.RP_ETA)
                    # v row vector [1, 256]
                    for ic in range(2):
                        pt = vpsum.tile([1, 128], dt.float32, tag="ptv")
                        nc.tensor.transpose(pt[:], v2_sb[:, side, ic, :], ident_sb[:])
                        nc.scalar.activation(vrow2[:, side, ic * 128:(ic + 1) * 128],
                                             pt[:], AFT.Copy)
                    # broadcast v^T to all partitions: ones^T @ vrow
                    pvb = vpsum.tile([128, F], dt.float32, tag="pvb")
                    nc.tensor.matmul(pvb[:], ones_sb[:], vrow2[:, side, :],
                                     start=True, stop=True)
                    nc.scalar.activation(vb2[:, side, :], pvb[:], AFT.Copy)
                    # wv^T = v^T W^T  [1, 256], broadcast to all partitions
                    pwv = vpsum.tile([1, F], dt.float32, tag="pwv")
                    for ic in range(2):
                        nc.tensor.matmul(pwv[:], v2_sb[:, side, ic, :], wt_sb[:, ic, :],
                                         start=(ic == 0), stop=(ic == 1))
                    wvrow = vpool.tile([1, F], dt.float32, tag="wvrow")
                    nc.scalar.activation(wvrow[:], pwv[:], AFT.Copy)
                    pwb = vpsum.tile([128, F], dt.float32, tag="pwb")
                    nc.tensor.matmul(pwb[:], ones_sb[:], wvrow[:],
                                     start=True, stop=True)
                    wvb = vpool.tile([128, F], dt.float32, tag="wvb")
                    nc.scalar.activation(wvb[:], pwb[:], AFT.Copy)
                    # W0[jc] = wt[jc] + (ncoef*v)[jc-part] * wv^T
                    for jc in range(2):
                        cv = vpool.tile([128, 1], dt.float32, tag="cv")
                        nc.vector.tensor_tensor(cv[:], v2_sb[:, side, jc, :],
                                                ncoef2[:, side, :], ALU.mult)
                        sc = vpool.tile([128, F], dt.float32, tag="sc")
                        nc.vector.tensor_scalar(sc[:], wvb[:], cv[:], None, ALU.mult)
                        nc.vector.tensor_add(w0_sb[:, side, jc, :], sc[:],
                                             wt_sb[:, jc, :])

            # == RP phase d: X' = X + ncoef (Xv) v^T (DVE), H0 = sign(X@W0) ==
            with tc.tile_pool(name="rpd", bufs=3) as dpool, \
                 tc.tile_pool(name="rpd_xt", bufs=3) as dxt, \
                 tc.tile_pool(name="rpd_pt", bufs=2, space="PSUM") as dpt, \
                 tc.tile_pool(name="rpd_ph", bufs=2, space="PSUM") as dph:
                for k in range(KPC):
                    side = 0 if k < cfg.U_KPC else 1
                    p = piece_of[k]
                    k0 = pieces[p][0]
                    xb = xst[:, k, :]
                    # u = X v (per-partition scalar), cu = ncoef * u
                    scr = dpool.tile([128, F], dt.float32, tag="scr")
                    nc.vector.tensor_mul(scr[:], xb, vb2[:, side, :])
                    u = dpool.tile([128, 1], dt.float32, tag="u")
                    nc.vector.tensor_reduce(u[:], scr[:],
                                            mybir.AxisListType.X, ALU.add)
                    cu = dpool.tile([128, 1], dt.float32, tag="cu")
                    nc.vector.tensor_tensor(cu[:], u[:], ncoef2[:, side, :], ALU.mult)
                    # X' = (vrow * cu) + X
                    scv = dpool.tile([128, F], dt.float32, tag="scv")
                    nc.vector.tensor_scalar(scv[:], vb2[:, side, :], cu[:], None,
                                            ALU.mult)
                    xs = dpool.tile([128, F], dt.float32, tag="xs")
                    nc.vector.tensor_add(xs[:], scv[:], xb)
                    xh = dpool.tile([128, F], dt.float16, tag="xh")
                    nc.scalar.activation(xh[:], xs[:], AFT.Copy)
                    nc.sync.dma_start(ag0[p][(k - k0) * 128:(k - k0 + 1) * 128, :], xh[:])
                    if xdbg is not None:
                        nc.sync.dma_start(xdbg[0][k * 128:(k + 1) * 128, :], xs[:])
                    # H0 = sign(X @ W0) via transposed X
                    xt = dxt.tile([128, 2, 128], dt.float32, tag="xt")
                    for c in range(2):
                        pt = dpt.tile([128, 128], dt.float32, tag="pt")
                        nc.tensor.transpose(pt[:], xst[:, k, c * 128:(c + 1) * 128], ident_sb[:])
                        nc.scalar.activation(xt[:, c, :], pt[:], AFT.Copy)
                    ph = dph.tile([128, F], dt.float32, tag="ph")
                    for jc in range(2):
                        nc.tensor.matmul(ph[:], xt[:, jc, :], w0_sb[:, side, jc, :],
                                         start=(jc == 0), stop=(jc == 1))
                    hb = dpool.tile([128, F], dt.int8, tag="hb")
                    nc.scalar.sign(hb[:], ph[:])
                    nc.sync.dma_start(out_d[k * 128:(k + 1) * 128, 0:F], hb[:])
                    # fire piece q a few blocks after its last input so the
                    # collective's input-wait never stalls dispatch
                    if k in fire_at:
                        fire_piece(ag0, tab0, fire_at[k])
            stash_cm.__exit__(None, None, None)

            # ======================== spmm layers ========================
            for L in (1, 2):
                tabs = tab0 if L == 1 else tab1
                with tc.tile_pool(name=f"l{L}_g", bufs=9) as gpool, \
                     tc.tile_pool(name=f"l{L}_s", bufs=3) as spool, \
                     tc.tile_pool(name=f"l{L}_x", bufs=3) as xpool, \
                     tc.tile_pool(name=f"l{L}_xt", bufs=3) as xtpool, \
                     tc.tile_pool(name=f"l{L}_px", bufs=2, space="PSUM") as pxp, \
                     tc.tile_pool(name=f"l{L}_pt", bufs=2, space="PSUM") as ptp, \
                     tc.tile_pool(name=f"l{L}_ph", bufs=2, space="PSUM") as php:
                    W = min(cfg.PREFETCH_W, len(groups))
                    gtiles = {}

                    def emit_gathers(gi, chunks):
                        grp_ks = groups[gi]
                        base = int(seg_off[grp_ks[0], 0])
                        if gi not in gtiles:
                            ntg = int(sum(tiles_pb[k] for k in grp_ks))
                            gtiles[gi] = gpool.tile([128, ntg, F], dt.float16,
                                                    tag="g", name=f"g{L}_{gi}")
                        g = gtiles[gi]
                        for c in chunks:
                            tgc = int(sum(T[k, c] for k in grp_ks))
                            if tgc == 0:
                                continue
                            goff = int(seg_off[grp_ks[0], c])
                            t0 = goff - base
                            tab_ap = tabs[c][:]
                            nc.gpsimd.dma_gather(
                                g[:, t0:t0 + tgc, :], tab_ap,
                                idx_sb[:, goff * 8:(goff + tgc) * 8],
                                tgc * 128, tgc * 128, F,
                                queue_num=(c if c < 2 else
                                           2 + (groups[gi][0] & 2) // 2))

                    # warm-up: chunk-0/1 gathers of the first W groups run
                    # while the last AllGather piece is still in flight
                    for gi in range(W):
                        emit_gathers(gi, range(NCH - 1))
                    for gi, grp_ks in enumerate(groups):
                        kg0 = grp_ks[0]
                        p = piece_of[kg0]
                        k0p = pieces[p][0]
                        base = int(seg_off[kg0, 0])       # first tile of group
                        ntg = int(sum(tiles_pb[k] for k in grp_ks))
                        if gi < W:
                            emit_gathers(gi, (NCH - 1,))
                        else:
                            emit_gathers(gi, range(NCH))
                        g = gtiles.pop(gi)
                        if cfg.S_ONFLY:
                            cmp = spool.tile([128, ntg, 128], dt.float16, tag="cmp")
                            nc.vector.tensor_tensor(
                                cmp[:], iotab_sb[:, 0:ntg, :],
                                rv_sb[:, 0, base:base + ntg].to_broadcast(
                                    [128, ntg, 128]),
                                ALU.is_equal)
                            s_blk = spool.tile([128, ntg, 128], dt.float16, tag="st")
                            nc.vector.tensor_tensor(
                                s_blk[:], cmp[:],
                                rv_sb[:, 1, base:base + ntg].to_broadcast(
                                    [128, ntg, 128]),
                                ALU.mult)
                        else:
                            s_blk = spool.tile([128, ntg, SC], dt.float16, tag="st")
                            nc.sync.dma_start(
                                s_blk[:], st_d[:, base * SC:(base + ntg) * SC])
                        for k in grp_ks:
                            px = pxp.tile([128, F], dt.float32, tag="px")
                            tsegs = [(int(seg_off[k, c]) - base, int(T[k, c]))
                                     for c in range(NCH)]
                            nseq = sum(n for _, n in tsegs)
                            cnt = 0
                            for t0k, ntc in tsegs:
                                for t in range(t0k, t0k + ntc):
                                    nc.tensor.matmul(
                                        px[:], s_blk[:, t, 0:128],
                                        g[:, t, :],
                                        start=(cnt == 0), stop=(cnt == nseq - 1))
                                    cnt += 1
                            x_sb = xpool.tile([128, F], dt.float32, tag="x")
                            nc.scalar.activation(x_sb[:], px[:], AFT.Copy)
                            if L == 1:
                                xh = xpool.tile([128, F], dt.float16, tag="xh")
                                nc.scalar.activation(xh[:], px[:], AFT.Copy)
                                nc.sync.dma_start(
                                    ag1[p][(k - k0p) * 128:(k - k0p + 1) * 128, :],
                                    xh[:])
                            if xdbg is not None:
                                nc.sync.dma_start(xdbg[L][k * 128:(k + 1) * 128, :],
                                                  x_sb[:])
                            xt = xtpool.tile([128, 2, 128], dt.float32, tag="xt")
                            for c in range(2):
                                pt = ptp.tile([128, 128], dt.float32, tag="pt")
                                nc.tensor.transpose(pt[:],
                                                    x_sb[:, c * 128:(c + 1) * 128],
                                                    ident_sb[:])
                                nc.scalar.activation(xt[:, c, :], pt[:], AFT.Copy)
                            ph = php.tile([128, F], dt.float32, tag="ph")
                            for jc in range(2):
                                nc.tensor.matmul(ph[:], xt[:, jc, :], wt_sb[:, jc, :],
                                                 start=(jc == 0), stop=(jc == 1))
                            hb = xpool.tile([128, F], dt.int8, tag="hb")
                            nc.scalar.sign(hb[:], ph[:])
                            nc.sync.dma_start(
                                out_d[k * 128:(k + 1) * 128, L * F:(L + 1) * F],
                                hb[:])
                        if L == 1 and grp_ks[-1] in fire_at:
                            fire_piece(ag1, tab1, fire_at[grp_ks[-1]])
    nc.compile()
    return nc


# --------------------------------------------------------------- runner
def _run(cfg, user_emb, item_emb, hash_W, rp_v0_user, rp_v0_item,
         edge_val, edge_row, edge_col, trace=False):
    prep = host_prep(cfg, user_emb, item_emb, edge_val, edge_row, edge_col)
    nc = build_program(cfg, prep)

    F = cfg.CON_DIM
    wt_np = np.ascontiguousarray(hash_W.T)
    v0_np = np.zeros((128, 2, 2, cfg.RP_AVG), np.float32)
    for side, v0 in ((0, rp_v0_user), (1, rp_v0_item)):
        v0_np[:, side, 0, :] = v0[0:128, :]
        v0_np[:, side, 1, :] = v0[128:256, :]
    v0_np = v0_np.reshape(128, -1)
    ident_np = np.eye(128, dtype=np.float32)
    ones_np = np.ones((1, 128), np.float32)

    maxt = max(int(sum(prep["tiles_per_block"][k] for k in grp_ks))
               for grp_ks in prep["groups"])
    iotab_np = np.ascontiguousarray(
        np.tile(np.arange(128, dtype=np.float16), (128, maxt)))
    in_maps = []
    for c in range(cfg.N_CORES):
        m = {
            "emb": np.ascontiguousarray(prep["emb_pc"][c]),
            "idx": np.ascontiguousarray(prep["idx_lay"][c]),
            "st": np.ascontiguousarray(prep["st_lay"][c]),
            "wt": wt_np, "v0": v0_np, "ident": ident_np, "ones1": ones_np,
        }
        if cfg.S_ONFLY:
            m["iotab"] = iotab_np
        in_maps.append(m)

    from concourse.bass_utils import run_bass_kernel_spmd
    res = run_bass_kernel_spmd(nc, in_maps, core_ids=list(range(cfg.N_CORES)),
                               trace=trace)

    full = np.concatenate([res.results[c]["out"] for c in range(cfg.N_CORES)],
                          axis=0)
    out = full[prep["local_of_node"]]
    return out, res


def kernel(user_emb, item_emb, hash_W, rp_v0_user, rp_v0_item,
           edge_val, edge_row, edge_col):
    cfg = Cfg()
    out, _ = _run(cfg, user_emb, item_emb, hash_W, rp_v0_user, rp_v0_item,
                  edge_val, edge_row, edge_col)
    return out.astype(np.float32)


# revision 86
# speedup vs baseline: 1.0014x; 1.0014x over previous
"""Trainium2 Bass kernel for nn_BGCH (2-layer GNN message passing + binary hash).

Computation (see reference):
  u = random_projection(user_emb, v0_u); i = random_projection(item_emb, v0_i)
  x0 = concat(u, i)                                  [90000, 256]
  H0 = sign(x0 @ W.T)
  x1 = spmm(A, x0); H1 = sign(x1 @ W.T)
  x2 = spmm(A, x1); H2 = sign(x2 @ W.T)
  out = concat([H0, H1, H2], axis=1)                 [90000, 768]

Distribution: nodes sharded over 8 cores (89 row-blocks of 128 per core),
edges partitioned by destination row. Per layer each core gathers x[col]
rows (fp16, 512B) from a replicated DRAM table via gpsimd.dma_gather
(int16 indices, 4 SWDGE queues in parallel), does a segmented-sum via
TensorE matmul with a host-precomputed fp16 S^T scatter matrix (one val
per edge slot at its dest-row column), then the dense hash locally in
fp32. The replicated table is rebuilt between layers by 3 piece-wise
AllGathers (pieces of 32 k-blocks = exactly one 32768-row gather chunk
in a piece-major slot layout), so collectives overlap compute and
chunk-c gathers start as soon as piece c lands. The random projection
is applied as a rank-1 update X' = X - c (Xv) v^T on DVE (v from a
Gram-matrix power iteration, Gram AllReduced across cores); the hash
weight is pre-projected the same way (W0 = W^T - (cv)(Wv)^T).
Numerics: fp16 table+vals cost ~4.8K sign flips of the ~6.9K budget
(rel_err 2e-2); hash stays fp32. S_HILO=True falls back to fp16 hi/lo
scatter vals (2 matmuls/tile) if more margin is needed.
"""
import sys
sys.path.insert(0, "/opt/trn_rl_repo")

import numpy as np


# ---------------------------------------------------------------- config
class Cfg:
    N_USERS = 30000
    N_ITEMS = 60000
    CON_DIM = 256
    N_LAYERS = 2
    RP_ITER = 2
    RP_ETA = 0.5
    RP_AVG = 3
    N_CORES = 8
    U_BLOCKS = 240          # user block slots (multiple of N_CORES)
    I_BLOCKS = 472          # item block slots (multiple of N_CORES)
    CHUNK = 32768           # int16 gather-index range
    PIECE_K = 32            # k-blocks per AllGather piece (32*8*128 = CHUNK)
    GMERGE = 1              # blocks per merged gather group (divides PIECE_K)
    PREFETCH_W = 8          # layer-start chunk-0/1 gather warm-up depth
    BAL_ITERS = 60000       # k-slot swap polish iterations (pad reduction)
    S_HILO = False          # True: fp16 hi/lo scatter vals (2 matmuls/tile)
    S_ONFLY = True          # build S^T tiles on DVE ((iota==rloc)*val)
    DEBUG_X = False

    @property
    def SCOLS(self):        # S^T columns per tile
        return 256 if self.S_HILO else 128

    @property
    def BLOCKS(self):
        return self.U_BLOCKS + self.I_BLOCKS

    @property
    def KPC(self):  # blocks per core
        return self.BLOCKS // self.N_CORES

    @property
    def U_KPC(self):
        return self.U_BLOCKS // self.N_CORES

    @property
    def N_SLOTS(self):
        return self.BLOCKS * 128

    @property
    def ROWS_PC(self):
        return self.KPC * 128

    @property
    def N_CHUNKS(self):
        return (self.N_SLOTS + self.CHUNK - 1) // self.CHUNK

    @property
    def PIECES(self):       # [(k0, k1)] per piece; pieces tile the CHUNKs
        out = []
        k = 0
        while k < self.KPC:
            out.append((k, min(k + self.PIECE_K, self.KPC)))
            k += self.PIECE_K
        return out


# ------------------------------------------------------------- host prep
def _pack_nodes(deg, nblocks, b0, rng):
    """Greedy bin-pack nodes into blocks of <=128 rows, balancing edge load.
    Returns (blk, rowin) arrays."""
    import heapq
    n = len(deg)
    order = np.argsort(-deg, kind="stable")
    heap = [(0, b0 + i, 0) for i in range(nblocks)]
    heapq.heapify(heap)
    blk = np.empty(n, np.int32)
    rowin = np.empty(n, np.int32)
    for node in order:
        while True:
            load, b, cnt = heapq.heappop(heap)
            if cnt < 128:
                break
            # block full: drop it from the heap permanently
        blk[node] = b
        rowin[node] = cnt
        heapq.heappush(heap, (load + int(deg[node]), b, cnt + 1))
    return blk, rowin


def host_prep(cfg, user_emb, item_emb, edge_val, edge_row, edge_col):
    """Node->slot permutation (piece-major table layout), block->core
    assignment, per-core edge tile template (uniform across cores) and
    packed device input arrays."""
    N = cfg.N_USERS + cfg.N_ITEMS
    rng = np.random.default_rng(0)
    deg = np.bincount(edge_row, minlength=N)

    ub, ur = _pack_nodes(deg[:cfg.N_USERS], cfg.U_BLOCKS, 0, rng)
    ib, ir = _pack_nodes(deg[cfg.N_USERS:], cfg.I_BLOCKS, cfg.U_BLOCKS, rng)
    blk = np.concatenate([ub, ib])
    rowin = np.concatenate([ur, ir])

    # block -> (core, k); user blocks get k in [0, U_KPC), items [U_KPC, KPC)
    bload = np.bincount(blk[edge_row], minlength=cfg.BLOCKS)
    core_of = np.empty(cfg.BLOCKS, np.int32)
    k_of = np.empty(cfg.BLOCKS, np.int32)
    for lo, hi, k0 in ((0, cfg.U_BLOCKS, 0), (cfg.U_BLOCKS, cfg.BLOCKS, cfg.U_KPC)):
        ids = np.arange(lo, hi)
        order = ids[np.argsort(-bload[lo:hi], kind="stable")]
        cnt = np.zeros(cfg.N_CORES, np.int32)
        for i, b in enumerate(order):
            rnd, pos = divmod(i, cfg.N_CORES)
            c = pos if rnd % 2 == 0 else cfg.N_CORES - 1 - pos
            core_of[b] = c
            k_of[b] = k0 + cnt[c]
            cnt[c] += 1

    # piece-major global slot: piece p holds all cores' k in [k0p, k1p),
    # core-major within the piece, so AllGather piece output == table piece
    pieces = cfg.PIECES
    pk0 = np.empty(cfg.KPC, np.int64)     # piece start k, per k
    pkn = np.empty(cfg.KPC, np.int64)     # piece size in blocks, per k
    pbase = np.empty(cfg.KPC, np.int64)   # piece base slot, per k
    base = 0
    for (k0, k1) in pieces:
        pk0[k0:k1] = k0
        pkn[k0:k1] = k1 - k0
        pbase[k0:k1] = base
        base += cfg.N_CORES * (k1 - k0) * 128

    def slots_of_all_nodes():
        c_, k_ = core_of[blk], k_of[blk]
        return pbase[k_] + (c_ * pkn[k_] + (k_ - pk0[k_])) * 128 + rowin

    # within each core+side, order k slots by chunk-0 edge count to reduce
    # cross-core template padding
    slot_of_node = slots_of_all_nodes()
    chunk_of_node = slot_of_node // cfg.CHUNK
    ccount = np.zeros((cfg.N_CORES, cfg.KPC, cfg.N_CHUNKS), np.int64)
    np.add.at(ccount, (core_of[blk[edge_row]], k_of[blk[edge_row]],
                       chunk_of_node[edge_col]), 1)
    for c in range(cfg.N_CORES):
        for lo, hi in ((0, cfg.U_KPC), (cfg.U_KPC, cfg.KPC)):
            seg = ccount[c, lo:hi]
            key = seg[:, 0] * 1000000 + seg[:, 1]
            perm = np.argsort(key, kind="stable")
            mask = (core_of == c) & (k_of >= lo) & (k_of < hi)
            bids = np.where(mask)[0]
            old_k = k_of[bids] - lo
            inv = np.empty_like(perm)
            inv[perm] = np.arange(len(perm))
            k_of[bids] = lo + inv[old_k]
            ccount[c, lo:hi] = seg[perm]

    slot_of_node = slots_of_all_nodes()
    chunk_of_node = slot_of_node // cfg.CHUNK
    local_of_node = core_of[blk] * cfg.ROWS_PC + k_of[blk] * 128 + rowin

    # recompute actual per-(core,k,chunk) counts after the k reorder
    ccount = np.zeros((cfg.N_CORES, cfg.KPC, cfg.N_CHUNKS), np.int64)
    np.add.at(ccount, (core_of[blk[edge_row]], k_of[blk[edge_row]],
                       chunk_of_node[edge_col]), 1)

    # local-search polish: swap k slots within (core, side) to shrink the
    # padded template sum_{k,c} ceil(max_core/128). NOTE: moving a block to
    # a different k changes its piece, hence its chunk for SOURCE purposes,
    # so only swap within the same piece (keeps chunk_of_node valid).
    bids = np.full((cfg.N_CORES, cfg.KPC), -1, np.int64)
    bids[core_of, k_of] = np.arange(cfg.BLOCKS)
    cc = ccount
    rng2 = np.random.default_rng(1)

    def col_tiles(k):
        return int(np.ceil(cc[:, k, :].max(axis=0) / 128.0).sum())

    # swap candidates: same core, same side, same piece
    cand = []
    for (pk0_, pk1_) in pieces:
        for lo, hi in ((0, cfg.U_KPC), (cfg.U_KPC, cfg.KPC)):
            a, b = max(lo, pk0_), min(hi, pk1_)
            if b - a >= 2:
                cand.append((a, b))
    for it in range(cfg.BAL_ITERS):
        c = it % cfg.N_CORES
        a, b = cand[rng2.integers(len(cand))]
        j1, j2 = rng2.integers(a, b, 2)
        if j1 == j2:
            continue
        before = col_tiles(j1) + col_tiles(j2)
        cc[c, [j1, j2], :] = cc[c, [j2, j1], :]
        if col_tiles(j1) + col_tiles(j2) > before:
            cc[c, [j1, j2], :] = cc[c, [j2, j1], :]
        else:
            b1, b2 = bids[c, j1], bids[c, j2]
            bids[c, j1], bids[c, j2] = b2, b1
            k_of[b1], k_of[b2] = j2, j1

    slot_of_node = slots_of_all_nodes()
    chunk_of_node = slot_of_node // cfg.CHUNK
    local_of_node = core_of[blk] * cfg.ROWS_PC + k_of[blk] * 128 + rowin
    ccount = np.zeros((cfg.N_CORES, cfg.KPC, cfg.N_CHUNKS), np.int64)
    np.add.at(ccount, (core_of[blk[edge_row]], k_of[blk[edge_row]],
                       chunk_of_node[edge_col]), 1)

    # template: tiles per (k, chunk) = ceil(max over cores / 128)
    T = np.ceil(ccount.max(axis=0) / 128).astype(np.int64)  # [KPC, N_CHUNKS]
    tiles_per_block = T.sum(axis=1)                          # [KPC]
    tot_tiles = int(tiles_per_block.sum())
    tot_slots = tot_tiles * 128

    # per-edge fields
    e_blk = blk[edge_row]
    e_core = core_of[e_blk]
    e_k = k_of[e_blk]
    e_rloc = rowin[edge_row]
    e_src = slot_of_node[edge_col]
    e_chunk = e_src // cfg.CHUNK
    e_idx = (e_src % cfg.CHUNK).astype(np.int16)

    # stream order (group, chunk, k): gathers merge the G blocks of a group
    # into one dma_gather per (group, chunk)
    groups = []
    for (k0, k1) in pieces:
        k = k0
        while k < k1:
            groups.append(list(range(k, min(k + cfg.GMERGE, k1))))
            k += cfg.GMERGE
    seg_rank = np.zeros((cfg.KPC, cfg.N_CHUNKS), np.int64)
    seg_tile_off = np.zeros((cfg.KPC, cfg.N_CHUNKS), np.int64)
    acc = 0
    rank = 0
    for grp_ks in groups:
        for c in range(cfg.N_CHUNKS):
            for k in grp_ks:
                seg_rank[k, c] = rank
                rank += 1
                seg_tile_off[k, c] = acc
                acc += T[k, c]

    idx_all = np.zeros((cfg.N_CORES, tot_slots), np.int16)
    rloc_all = np.zeros((cfg.N_CORES, tot_slots), np.float32)
    val_all = np.zeros((cfg.N_CORES, tot_slots), np.float32)
    e_rank = seg_rank[e_k, e_chunk]
    order = np.lexsort((e_rank, e_core))
    eo_core = e_core[order]
    eo_rank = e_rank[order]
    grp = eo_core.astype(np.int64) * (cfg.KPC * cfg.N_CHUNKS) + eo_rank
    uniq, first = np.unique(grp, return_index=True)
    within = np.arange(len(grp)) - np.repeat(first, np.diff(np.append(first, len(grp))))
    pos = seg_tile_off[e_k[order], e_chunk[order]] * 128 + within
    idx_all[eo_core, pos] = e_idx[order]
    rloc_all[eo_core, pos] = e_rloc[order].astype(np.float32)
    val_all[eo_core, pos] = edge_val[order]

    # device layouts
    idx_lay = np.zeros((cfg.N_CORES, 128, tot_slots // 16), np.int16)
    wrap = idx_all.reshape(cfg.N_CORES, -1, 16)
    for rep in range(8):
        idx_lay[:, rep * 16:(rep + 1) * 16, :] = wrap.transpose(0, 2, 1)

    # S^T tiles: one nonzero per partition (edge slot) at column rloc
    # (dest row within block).
    if cfg.S_ONFLY:
        # device builds S = (iota == rloc) * val on DVE; ship only the
        # per-(partition, tile) rloc and val planes (fp16)
        rv_lay = np.stack([
            rloc_all.reshape(cfg.N_CORES, tot_tiles, 128).transpose(0, 2, 1),
            val_all.reshape(cfg.N_CORES, tot_tiles, 128).transpose(0, 2, 1),
        ], axis=2)  # [cores, 128, 2, tot_tiles]
        st_lay = np.ascontiguousarray(
            rv_lay.reshape(cfg.N_CORES, 128, 2 * tot_tiles).astype(np.float16))
    else:
        SC = cfg.SCOLS
        v_hi = val_all.astype(np.float16)
        st_lay = np.zeros((cfg.N_CORES, 128, tot_tiles * SC), np.float16)
        cidx = np.repeat(np.arange(cfg.N_CORES), tot_slots)
        pp_ = np.tile(np.arange(tot_slots) % 128, cfg.N_CORES)
        tt_ = np.tile(np.arange(tot_slots) // 128, cfg.N_CORES)
        rl_ = rloc_all.reshape(-1).astype(np.int64)
        st_lay[cidx, pp_, tt_ * SC + rl_] = v_hi.reshape(-1)
        if cfg.S_HILO:
            v_lo = (val_all - v_hi.astype(np.float32)).astype(np.float16)
            st_lay[cidx, pp_, tt_ * SC + 128 + rl_] = v_lo.reshape(-1)

    # packed embeddings per core (local k-major layout)
    emb_all = np.zeros((cfg.N_CORES * cfg.ROWS_PC, cfg.CON_DIM), np.float32)
    emb_all[local_of_node[:cfg.N_USERS]] = user_emb
    emb_all[local_of_node[cfg.N_USERS:]] = item_emb
    emb_pc = emb_all.reshape(cfg.N_CORES, cfg.ROWS_PC, cfg.CON_DIM)

    return dict(
        local_of_node=local_of_node, T=T, tiles_per_block=tiles_per_block,
        seg_tile_off=seg_tile_off, tot_tiles=tot_tiles, tot_slots=tot_slots,
        idx_lay=idx_lay, st_lay=st_lay, emb_pc=emb_pc, groups=groups,
    )


# ------------------------------------------------------------ bass build
def build_program(cfg, prep):
    import concourse.bacc as bacc
    import concourse.mybir as mybir
    import concourse.tile as tile

    dt = mybir.dt
    F = cfg.CON_DIM
    KPC = cfg.KPC
    T = prep["T"]
    seg_off = prep["seg_tile_off"]
    tiles_pb = prep["tiles_per_block"]
    tot_tiles = prep["tot_tiles"]
    tot_slots = prep["tot_slots"]
    NCH = cfg.N_CHUNKS
    SC = cfg.SCOLS
    pieces = cfg.PIECES
    groups = prep["groups"]
    AFT = mybir.ActivationFunctionType
    ALU = mybir.AluOpType

    nc = bacc.Bacc("TRN2", target_bir_lowering=False, debug=False,
                   num_devices=cfg.N_CORES, num_swdge_queues=4)

    emb_d = nc.dram_tensor("emb", [cfg.ROWS_PC, F], dt.float32, kind="ExternalInput")
    idx_d = nc.dram_tensor("idx", [128, tot_slots // 16], dt.int16, kind="ExternalInput")
    MAXT = max(int(sum(tiles_pb[k] for k in grp_ks)) for grp_ks in groups)
    if cfg.S_ONFLY:
        st_d = nc.dram_tensor("st", [128, 2 * tot_tiles], dt.float16,
                              kind="ExternalInput")
        iotab_d = nc.dram_tensor("iotab", [128, MAXT * 128], dt.float16,
                                 kind="ExternalInput")
    else:
        st_d = nc.dram_tensor("st", [128, tot_tiles * SC], dt.float16,
                              kind="ExternalInput")
    wt_d = nc.dram_tensor("wt", [F, F], dt.float32, kind="ExternalInput")    # W^T
    v0_d = nc.dram_tensor("v0", [128, 2 * 2 * cfg.RP_AVG], dt.float32, kind="ExternalInput")
    ident_d = nc.dram_tensor("ident", [128, 128], dt.float32, kind="ExternalInput")
    ones_d = nc.dram_tensor("ones1", [1, 128], dt.float32, kind="ExternalInput")
    out_d = nc.dram_tensor("out", [cfg.ROWS_PC, 3 * F], dt.int8, kind="ExternalOutput")
    xdbg = None
    if cfg.DEBUG_X:
        xdbg = [nc.dram_tensor(f"xdbg{i}", [cfg.ROWS_PC, F], dt.float32,
                               kind="ExternalOutput") for i in range(3)]

    # internal DRAM: fp16 x tables, piece-major. One tab tensor per gather
    # CHUNK; each AllGather piece writes its row-slice of its chunk's tab.
    piece_rows = [cfg.N_CORES * (k1 - k0) * 128 for (k0, k1) in pieces]
    piece_base = np.concatenate([[0], np.cumsum(piece_rows)])

    def piece_tensors(nm):
        ag, tab = [], []
        for p, (k0, k1) in enumerate(pieces):
            ag.append(nc.dram_tensor(f"{nm}_ag{p}", [(k1 - k0) * 128, F],
                                     dt.float16))
        for c in range(NCH):
            rows = min(cfg.N_SLOTS, (c + 1) * cfg.CHUNK) - c * cfg.CHUNK
            tab.append(nc.dram_tensor(f"{nm}_tab{c}", [rows, F], dt.float16,
                                      addr_space="Shared"))
        return ag, tab

    ag0, tab0 = piece_tensors("t0")
    ag1, tab1 = piece_tensors("t1")
    gr_in = nc.dram_tensor("gr_in", [4 * 128, F], dt.float32)
    gr_out = nc.dram_tensor("gr_out", [4 * 128, F], dt.float32)

    rg = [list(range(cfg.N_CORES))]
    piece_of = {}
    for q, (k0q, k1q) in enumerate(pieces):
        for kk in range(k0q, k1q):
            piece_of[kk] = q
    # piece p's inputs complete at block k1-1; fire 3 blocks later (capped)
    fire_at = {min(k1 - 1 + 3, cfg.KPC - 1): q
               for q, (k0, k1) in enumerate(pieces)}

    def fire_piece(ag, tab, p):
        r0 = int(piece_base[p])
        c = r0 // cfg.CHUNK
        off = r0 - c * cfg.CHUNK
        nc.gpsimd.collective_compute(
            "AllGather", ALU.bypass, replica_groups=rg,
            ins=[ag[p].ap().opt()],
            outs=[tab[c][off:off + piece_rows[p], :].opt()])

    with tile.TileContext(nc) as tc:
        with tc.tile_pool(name="const", bufs=1) as cpool:
            # preload the gpsimd library holding DMAGatherAnt so the ~60us
            # reload overlaps the Gram phase instead of stalling layer 1
            from concourse import library_config
            nc.gpsimd.load_library(library_config.mlp)
            ident_sb = cpool.tile([128, 128], dt.float32, tag="ident")
            nc.sync.dma_start(ident_sb[:], ident_d[:])
            ones_sb = cpool.tile([1, 128], dt.float32, tag="ones1")
            nc.sync.dma_start(ones_sb[:], ones_d[:])
            wt_sb = cpool.tile([128, 2, F], dt.float32, tag="wt")
            nc.sync.dma_start(wt_sb[:, 0, :], wt_d[0:128, :])
            nc.sync.dma_start(wt_sb[:, 1, :], wt_d[128:256, :])
            v0_sb = cpool.tile([128, 2, 2, cfg.RP_AVG], dt.float32, tag="v0")
            nc.sync.dma_start(v0_sb[:], v0_d[:])
            idx_sb = cpool.tile([128, tot_slots // 16], dt.int16, tag="idx")
            nc.sync.dma_start(idx_sb[:], idx_d[:])
            if cfg.S_ONFLY:
                rv_sb = cpool.tile([128, 2, tot_tiles], dt.float16, tag="rv")
                nc.sync.dma_start(rv_sb[:], st_d[:])
                iotab_sb = cpool.tile([128, MAXT, 128], dt.float16, tag="iotab")
                nc.sync.dma_start(iotab_sb[:], iotab_d[:])

            # =================== RP phase a: Gram matrices ===================
            # emb is DMAed once into a persistent stash; phase d reads it
            # from SBUF (stash pool closed before the layers)
            stash_cm = tc.tile_pool(name="stash", bufs=1)
            stpool = stash_cm.__enter__()
            xst = stpool.tile([128, KPC, F], dt.float32, tag="xst")
            with tc.tile_pool(name="rpa_ps", bufs=1, space="PSUM") as gpsum:
                pg = [gpsum.tile([128, F], dt.float32, tag=f"g{i}", name=f"pg{i}") for i in range(4)]
                # pg[0..1] = Gram_u chunks, pg[2..3] = Gram_i chunks
                # symmetric Gram: row-chunk 0 computes [G00|G01] fully; for
                # chunk 1 only G11 (G10 = G01^T, rebuilt after the AllReduce).
                # fp16 hi/lo operands (3-term product) instead of slow fp32.
                hp_cm = tc.tile_pool(name="rpa_h", bufs=4)
                hp = hp_cm.__enter__()
                for k in range(KPC):
                    nc.sync.dma_start(xst[:, k, :], emb_d[k * 128:(k + 1) * 128, :])
                    side = 0 if k < cfg.U_KPC else 1
                    first = k == 0 or k == cfg.U_KPC
                    last = k == cfg.U_KPC - 1 or k == KPC - 1
                    xhi = hp.tile([128, F], dt.float16, tag="xhi")
                    nc.scalar.activation(xhi[:], xst[:, k, :], AFT.Copy)
                    xh32 = hp.tile([128, F], dt.float32, tag="xh32")
                    nc.scalar.activation(xh32[:], xhi[:], AFT.Copy)
                    xlo = hp.tile([128, F], dt.float16, tag="xlo")
                    nc.vector.tensor_sub(xlo[:], xst[:, k, :], xh32[:])
                    terms = ((xhi, xhi), (xhi, xlo), (xlo, xhi))
                    for ti, (a, b) in enumerate(terms):
                        nc.tensor.matmul(pg[2 * side][:], a[:, 0:128], b[:],
                                         start=(first and ti == 0),
                                         stop=(last and ti == 2))
                        nc.tensor.matmul(pg[2 * side + 1][:, 128:256],
                                         a[:, 128:256], b[:, 128:256],
                                         start=(first and ti == 0),
                                         stop=(last and ti == 2))
                hp_cm.__exit__(None, None, None)
                g_sb = cpool.tile([128, 4, F], dt.float32, tag="gsb")
                for i in range(4):
                    if i % 2 == 0:
                        nc.scalar.activation(g_sb[:, i, :], pg[i][:], AFT.Copy)
                    else:
                        nc.scalar.activation(g_sb[:, i, 128:256],
                                             pg[i][:, 128:256], AFT.Copy)
                        nc.vector.memset(g_sb[:, i, 0:128], 0)
                for i in range(4):
                    nc.sync.dma_start(gr_in[i * 128:(i + 1) * 128, :], g_sb[:, i, :])
            nc.gpsimd.collective_compute(
                "AllReduce", ALU.add, replica_groups=rg,
                ins=[gr_in.ap().opt()], outs=[gr_out.ap().opt()])

            # ====== RP phase b/c: v, coef, W0 = W^T - (c v)(W v)^T per side ======
            vrow2 = cpool.tile([1, 2, F], dt.float32, tag="vrow2")      # v^T per side
            vb2 = cpool.tile([128, 2, F], dt.float32, tag="vb2")        # bcast v^T
            ncoef2 = cpool.tile([128, 2, 1], dt.float32, tag="ncoef2")  # -eta/vTv
            w0_sb = cpool.tile([128, 2, 2, F], dt.float32, tag="w0sb")
            v2_sb = cpool.tile([128, 2, 2, 1], dt.float32, tag="v2")    # [.,side,jc,.]
            with tc.tile_pool(name="rpb", bufs=1) as vpool, \
                 tc.tile_pool(name="rpb_ps", bufs=1, space="PSUM") as vpsum:
                gg = vpool.tile([128, 4, F], dt.float32, tag="gg")
                for i in range(4):
                    nc.sync.dma_start(gg[:, i, :], gr_out[i * 128:(i + 1) * 128, :])
                # rebuild G10 = G01^T in the row-chunk-1 slots
                for side in range(2):
                    ptg = vpsum.tile([128, 128], dt.float32, tag="pvb")
                    nc.tensor.transpose(ptg[:], gg[:, 2 * side, 128:256],
                                        ident_sb[:])
                    nc.scalar.activation(gg[:, 2 * side + 1, 0:128], ptg[:],
                                         AFT.Copy)
                for side in range(2):
                    cur = None
                    for it in range(cfg.RP_ITER):
                        pv = [vpsum.tile([128, cfg.RP_AVG], dt.float32, tag=f"pv{ic}",
                                        name=f"pv{ic}") for ic in range(2)]
                        for ic in range(2):
                            for kc in range(2):
                                rhs_ap = (v0_sb[:, side, kc, :] if cur is None
                                          else cur[:, kc, :])
                                nc.tensor.matmul(
                                    pv[ic][:],
                                    gg[:, 2 * side + kc, ic * 128:(ic + 1) * 128],
                                    rhs_ap,
                                    start=(kc == 0), stop=(kc == 1))
                        nxt = vpool.tile([128, 2, cfg.RP_AVG], dt.float32, tag=f"vk{it}")
                        for ic in range(2):
                            nc.scalar.activation(nxt[:, ic, :], pv[ic][:], AFT.Copy)
                        cur = nxt
                    # v = mean over restarts
                    for ic in range(2):
                        vsum = vpool.tile([128, 1], dt.float32, tag="vs")
                        nc.vector.tensor_reduce(vsum[:], cur[:, ic, :],
                                                mybir.AxisListType.X, ALU.add)
                        nc.scalar.activation(v2_sb[:, side, ic, :], vsum[:], AFT.Copy,
                                             scale=1.0 / cfg.RP_AVG)
                    # vTv
                    pn = vpsum.tile([1, 1], dt.float32, tag="pn")
                    for ic in range(2):
                        nc.tensor.matmul(pn[:], v2_sb[:, side, ic, :], v2_sb[:, side, ic, :],
                                         start=(ic == 0), stop=(ic == 1))
                    recip = vpool.tile([1, 1], dt.float32, tag="rec")
                    nc.vector.reciprocal(recip[:], pn[:])
                    # broadcast -eta/vTv to all partitions
                    pb = vpsum.tile([128, 1], dt.float32, tag="pb")
                    nc.tensor.matmul(pb[:], ones_sb[:], recip[:], start=True, stop=True)
                    nc.scalar.activation(ncoef2[:, side, :], pb[:], AFT.Copy,
                                         scale=-cfg.RP_ETA)
                    # v row vector [1, 256]
                    for ic in range(2):
                        pt = vpsum.tile([1, 128], dt.float32, tag="ptv")
                        nc.tensor.transpose(pt[:], v2_sb[:, side, ic, :], ident_sb[:])
                        nc.scalar.activation(vrow2[:, side, ic * 128:(ic + 1) * 128],
                                             pt[:], AFT.Copy)
                    # broadcast v^T to all partitions: ones^T @ vrow
                    pvb = vpsum.tile([128, F], dt.float32, tag="pvb")
                    nc.tensor.matmul(pvb[:], ones_sb[:], vrow2[:, side, :],
                                     start=True, stop=True)
                    nc.scalar.activation(vb2[:, side, :], pvb[:], AFT.Copy)
                    # wv^T = v^T W^T  [1, 256], broadcast to all partitions
                    pwv = vpsum.tile([1, F], dt.float32, tag="pwv")
                    for ic in range(2):
                        nc.tensor.matmul(pwv[:], v2_sb[:, side, ic, :], wt_sb[:, ic, :],
                                         start=(ic == 0), stop=(ic == 1))
                    wvrow = vpool.tile([1, F], dt.float32, tag="wvrow")
                    nc.scalar.activation(wvrow[:], pwv[:], AFT.Copy)
                    pwb = vpsum.tile([128, F], dt.float32, tag="pwb")
                    nc.tensor.matmul(pwb[:], ones_sb[:], wvrow[:],
                                     start=True, stop=True)
                    wvb = vpool.tile([128, F], dt.float32, tag="wvb")
                    nc.scalar.activation(wvb[:], pwb[:], AFT.Copy)
                    # W0[jc] = wt[jc] + (ncoef*v)[jc-part] * wv^T
                    for jc in range(2):
                        cv = vpool.tile([128, 1], dt.float32, tag="cv")
                        nc.vector.tensor_tensor(cv[:], v2_sb[:, side, jc, :],
                                                ncoef2[:, side, :], ALU.mult)
                        sc = vpool.tile([128, F], dt.float32, tag="sc")
                        nc.vector.tensor_scalar(sc[:], wvb[:], cv[:], None, ALU.mult)
                        nc.vector.tensor_add(w0_sb[:, side, jc, :], sc[:],
                                             wt_sb[:, jc, :])

            # == RP phase d: X' = X + ncoef (Xv) v^T (DVE), H0 = sign(X@W0) ==
            with tc.tile_pool(name="rpd", bufs=3) as dpool, \
                 tc.tile_pool(name="rpd_xt", bufs=3) as dxt, \
                 tc.tile_pool(name="rpd_pt", bufs=2, space="PSUM") as dpt, \
                 tc.tile_pool(name="rpd_ph", bufs=2, space="PSUM") as dph:
                for k in range(KPC):
                    side = 0 if k < cfg.U_KPC else 1
                    p = piece_of[k]
                    k0 = pieces[p][0]
                    xb = xst[:, k, :]
                    # u = X v (per-partition scalar), cu = ncoef * u
                    scr = dpool.tile([128, F], dt.float32, tag="scr")
                    nc.vector.tensor_mul(scr[:], xb, vb2[:, side, :])
                    u = dpool.tile([128, 1], dt.float32, tag="u")
                    nc.vector.tensor_reduce(u[:], scr[:],
                                            mybir.AxisListType.X, ALU.add)
                    cu = dpool.tile([128, 1], dt.float32, tag="cu")
                    nc.vector.tensor_tensor(cu[:], u[:], ncoef2[:, side, :], ALU.mult)
                    # X' = (vrow * cu) + X
                    scv = dpool.tile([128, F], dt.float32, tag="scv")
                    nc.vector.tensor_scalar(scv[:], vb2[:, side, :], cu[:], None,
                                            ALU.mult)
                    xs = dpool.tile([128, F], dt.float32, tag="xs")
                    nc.vector.tensor_add(xs[:], scv[:], xb)
                    xh = dpool.tile([128, F], dt.float16, tag="xh")
                    nc.scalar.activation(xh[:], xs[:], AFT.Copy)
                    nc.sync.dma_start(ag0[p][(k - k0) * 128:(k - k0 + 1) * 128, :], xh[:])
                    if xdbg is not None:
                        nc.sync.dma_start(xdbg[0][k * 128:(k + 1) * 128, :], xs[:])
                    # H0 = sign(X @ W0) via transposed X
                    xt = dxt.tile([128, 2, 128], dt.float32, tag="xt")
                    for c in range(2):
                        pt = dpt.tile([128, 128], dt.float32, tag="pt")
                        nc.tensor.transpose(pt[:], xst[:, k, c * 128:(c + 1) * 128], ident_sb[:])
                        nc.scalar.activation(xt[:, c, :], pt[:], AFT.Copy)
                    ph = dph.tile([128, F], dt.float32, tag="ph")
                    for jc in range(2):
                        nc.tensor.matmul(ph[:], xt[:, jc, :], w0_sb[:, side, jc, :],
                                         start=(jc == 0), stop=(jc == 1))
                    hb = dpool.tile([128, F], dt.int8, tag="hb")
                    nc.scalar.sign(hb[:], ph[:])
                    nc.sync.dma_start(out_d[k * 128:(k + 1) * 128, 0:F], hb[:])
                    # fire piece q a few blocks after its last input so the
                    # collective's input-wait never stalls dispatch
                    if k in fire_at:
                        fire_piece(ag0, tab0, fire_at[k])
            stash_cm.__exit__(None, None, None)

            # ======================== spmm layers ========================
            for L in (1, 2):
                tabs = tab0 if L == 1 else tab1
                with tc.tile_pool(name=f"l{L}_g", bufs=9) as gpool, \
                     tc.tile_pool(name=f"l{L}_s", bufs=3) as spool, \
                     tc.tile_pool(name=f"l{L}_x", bufs=3) as xpool, \
                     tc.tile_pool(name=f"l{L}_xt", bufs=3) as xtpool, \
                     tc.tile_pool(name=f"l{L}_px", bufs=2, space="PSUM") as pxp, \
                     tc.tile_pool(name=f"l{L}_pt", bufs=2, space="PSUM") as ptp, \
                     tc.tile_pool(name=f"l{L}_ph", bufs=2, space="PSUM") as php:
                    W = min(cfg.PREFETCH_W, len(groups))
                    gtiles = {}

                    def emit_gathers(gi, chunks):
                        grp_ks = groups[gi]
                        base = int(seg_off[grp_ks[0], 0])
                        if gi not in gtiles:
                            ntg = int(sum(tiles_pb[k] for k in grp_ks))
                            gtiles[gi] = gpool.tile([128, ntg, F], dt.float16,
                                                    tag="g", name=f"g{L}_{gi}")
                        g = gtiles[gi]
                        for c in chunks:
                            tgc = int(sum(T[k, c] for k in grp_ks))
                            if tgc == 0:
                                continue
                            goff = int(seg_off[grp_ks[0], c])
                            t0 = goff - base
                            tab_ap = tabs[c][:]
                            nc.gpsimd.dma_gather(
                                g[:, t0:t0 + tgc, :], tab_ap,
                                idx_sb[:, goff * 8:(goff + tgc) * 8],
                                tgc * 128, tgc * 128, F,
                                queue_num=(c if c < 2 else
                                           2 + (groups[gi][0] & 2) // 2))

                    # warm-up: chunk-0/1 gathers of the first W groups run
                    # while the last AllGather piece is still in flight
                    for gi in range(W):
                        emit_gathers(gi, range(NCH - 1))
                    for gi, grp_ks in enumerate(groups):
                        kg0 = grp_ks[0]
                        p = piece_of[kg0]
                        k0p = pieces[p][0]
                        base = int(seg_off[kg0, 0])       # first tile of group
                        ntg = int(sum(tiles_pb[k] for k in grp_ks))
                        if gi < W:
                            emit_gathers(gi, (NCH - 1,))
                        else:
                            emit_gathers(gi, range(NCH))
                        g = gtiles.pop(gi)
                        if cfg.S_ONFLY:
                            cmp = spool.tile([128, ntg, 128], dt.float16, tag="cmp")
                            nc.vector.tensor_tensor(
                                cmp[:], iotab_sb[:, 0:ntg, :],
                                rv_sb[:, 0, base:base + ntg].to_broadcast(
                                    [128, ntg, 128]),
                                ALU.is_equal)
                            s_blk = spool.tile([128, ntg, 128], dt.float16, tag="st")
                            nc.vector.tensor_tensor(
                                s_blk[:], cmp[:],
                                rv_sb[:, 1, base:base + ntg].to_broadcast(
                                    [128, ntg, 128]),
                                ALU.mult)
                        else:
                            s_blk = spool.tile([128, ntg, SC], dt.float16, tag="st")
                            nc.sync.dma_start(
                                s_blk[:], st_d[:, base * SC:(base + ntg) * SC])
                        for k in grp_ks:
                            px = pxp.tile([128, F], dt.float32, tag="px")
                            tsegs = [(int(seg_off[k, c]) - base, int(T[k, c]))
                                     for c in range(NCH)]
                            nseq = sum(n for _, n in tsegs)
                            cnt = 0
                            for t0k, ntc in tsegs:
                                for t in range(t0k, t0k + ntc):
                                    nc.tensor.matmul(
                                        px[:], s_blk[:, t, 0:128],
                                        g[:, t, :],
                                        start=(cnt == 0), stop=(cnt == nseq - 1))
                                    cnt += 1
                            x_sb = xpool.tile([128, F], dt.float32, tag="x")
                            nc.scalar.activation(x_sb[:], px[:], AFT.Copy)
                            if L == 1:
                                xh = xpool.tile([128, F], dt.float16, tag="xh")
                                nc.scalar.activation(xh[:], px[:], AFT.Copy)
                                nc.sync.dma_start(
                                    ag1[p][(k - k0p) * 128:(k - k0p + 1) * 128, :],
                                    xh[:])
                            if xdbg is not None:
                                nc.sync.dma_start(xdbg[L][k * 128:(k + 1) * 128, :],
                                                  x_sb[:])
                            xt = xtpool.tile([128, 2, 128], dt.float32, tag="xt")
                            for c in range(2):
                                pt = ptp.tile([128, 128], dt.float32, tag="pt")
                                nc.tensor.transpose(pt[:],
                                                    x_sb[:, c * 128:(c + 1) * 128],
                                                    ident_sb[:])
                                nc.scalar.activation(xt[:, c, :], pt[:], AFT.Copy)
                            ph = php.tile([128, F], dt.float32, tag="ph")
                            for jc in range(2):
                                nc.tensor.matmul(ph[:], xt[:, jc, :], wt_sb[:, jc, :],
                                                 start=(jc == 0), stop=(jc == 1))
                            hb = xpool.tile([128, F], dt.int8, tag="hb")
                            nc.scalar.sign(hb[:], ph[:])
                            nc.sync.dma_start(
                                out_d[k * 128:(k + 1) * 128, L * F:(L + 1) * F],
                                hb[:])
                        if L == 1 and grp_ks[-1] in fire_at:
                            fire_piece(ag1, tab1, fire_at[grp_ks[-1]])
    nc.compile()
    return nc


# --------------------------------------------------------------- runner
def _run(cfg, user_emb, item_emb, hash_W, rp_v0_user, rp_v0_item,
         edge_val, edge_row, edge_col, trace=False):
    prep = host_prep(cfg, user_emb, item_emb, edge_val, edge_row, edge_col)
    nc = build_program(cfg, prep)

    F = cfg.CON_DIM
    wt_np = np.ascontiguousarray(hash_W.T)
    v0_np = np.zeros((128, 2, 2, cfg.RP_AVG), np.float32)
    for side, v0 in ((0, rp_v0_user), (1, rp_v0_item)):
        v0_np[:, side, 0, :] = v0[0:128, :]
        v0_np[:, side, 1, :] = v0[128:256, :]
    v0_np = v0_np.reshape(128, -1)
    ident_np = np.eye(128, dtype=np.float32)
    ones_np = np.ones((1, 128), np.float32)

    maxt = max(int(sum(prep["tiles_per_block"][k] for k in grp_ks))
               for grp_ks in prep["groups"])
    iotab_np = np.ascontiguousarray(
        np.tile(np.arange(128, dtype=np.float16), (128, maxt)))
    in_maps = []
    for c in range(cfg.N_CORES):
        m = {
            "emb": np.ascontiguousarray(prep["emb_pc"][c]),
            "idx": np.ascontiguousarray(prep["idx_lay"][c]),
            "st": np.ascontiguousarray(prep["st_lay"][c]),
            "wt": wt_np, "v0": v0_np, "ident": ident_np, "ones1": ones_np,
        }
        if cfg.S_ONFLY:
            m["iotab"] = iotab_np
        in_maps.append(m)

    from concourse.bass_utils import run_bass_kernel_spmd
    res = run_bass_kernel_spmd(nc, in_maps, core_ids=list(range(cfg.N_CORES)),
                               trace=trace)

    full = np.concatenate([res.results[c]["out"] for c in range(cfg.N_CORES)],
                          axis=0)
    out = full[prep["local_of_node"]]
    return out, res


def kernel(user_emb, item_emb, hash_W, rp_v0_user, rp_v0_item,
           edge_val, edge_row, edge_col):
    cfg = Cfg()
    out, _ = _run(cfg, user_emb, item_emb, hash_W, rp_v0_user, rp_v0_item,
                  edge_val, edge_row, edge_col)
    return out.astype(np.float32)


# revision 88
# speedup vs baseline: 1.0262x; 1.0247x over previous
"""Trainium2 Bass kernel for nn_BGCH (2-layer GNN message passing + binary hash).

Computation (see reference):
  u = random_projection(user_emb, v0_u); i = random_projection(item_emb, v0_i)
  x0 = concat(u, i)                                  [90000, 256]
  H0 = sign(x0 @ W.T)
  x1 = spmm(A, x0); H1 = sign(x1 @ W.T)
  x2 = spmm(A, x1); H2 = sign(x2 @ W.T)
  out = concat([H0, H1, H2], axis=1)                 [90000, 768]

Distribution: nodes sharded over 8 cores (89 row-blocks of 128 per core),
edges partitioned by destination row. Per layer each core gathers x[col]
rows (fp16, 512B) from a replicated DRAM table via gpsimd.dma_gather
(int16 indices, 4 SWDGE queues in parallel), does a segmented-sum via
TensorE matmul with a host-precomputed fp16 S^T scatter matrix (one val
per edge slot at its dest-row column), then the dense hash locally in
fp32. The replicated table is rebuilt between layers by 3 piece-wise
AllGathers (pieces of 32 k-blocks = exactly one 32768-row gather chunk
in a piece-major slot layout), so collectives overlap compute and
chunk-c gathers start as soon as piece c lands. The random projection
is applied as a rank-1 update X' = X - c (Xv) v^T on DVE (v from a
Gram-matrix power iteration, Gram AllReduced across cores); the hash
weight is pre-projected the same way (W0 = W^T - (cv)(Wv)^T).
Numerics: fp16 table+vals cost ~4.8K sign flips of the ~6.9K budget
(rel_err 2e-2); hash stays fp32. S_HILO=True falls back to fp16 hi/lo
scatter vals (2 matmuls/tile) if more margin is needed.
"""
import sys
sys.path.insert(0, "/opt/trn_rl_repo")

import numpy as np


# ---------------------------------------------------------------- config
class Cfg:
    N_USERS = 30000
    N_ITEMS = 60000
    CON_DIM = 256
    N_LAYERS = 2
    RP_ITER = 2
    RP_ETA = 0.5
    RP_AVG = 3
    N_CORES = 8
    U_BLOCKS = 240          # user block slots (multiple of N_CORES)
    I_BLOCKS = 472          # item block slots (multiple of N_CORES)
    CHUNK = 32768           # int16 gather-index range
    PIECE_K = 32            # k-blocks per AllGather piece (32*8*128 = CHUNK)
    GMERGE = 1              # blocks per merged gather group (divides PIECE_K)
    PREFETCH_W = 11          # layer-start chunk-0/1 gather warm-up depth
    BAL_ITERS = 60000       # k-slot swap polish iterations (pad reduction)
    S_HILO = False          # True: fp16 hi/lo scatter vals (2 matmuls/tile)
    S_ONFLY = True          # build S^T tiles on DVE ((iota==rloc)*val)
    DEBUG_X = False

    @property
    def SCOLS(self):        # S^T columns per tile
        return 256 if self.S_HILO else 128

    @property
    def BLOCKS(self):
        return self.U_BLOCKS + self.I_BLOCKS

    @property
    def KPC(self):  # blocks per core
        return self.BLOCKS // self.N_CORES

    @property
    def U_KPC(self):
        return self.U_BLOCKS // self.N_CORES

    @property
    def N_SLOTS(self):
        return self.BLOCKS * 128

    @property
    def ROWS_PC(self):
        return self.KPC * 128

    @property
    def N_CHUNKS(self):
        return (self.N_SLOTS + self.CHUNK - 1) // self.CHUNK

    @property
    def PIECES(self):       # [(k0, k1)] per piece; pieces tile the CHUNKs
        out = []
        k = 0
        while k < self.KPC:
            out.append((k, min(k + self.PIECE_K, self.KPC)))
            k += self.PIECE_K
        return out


# ------------------------------------------------------------- host prep
def _pack_nodes(deg, nblocks, b0, rng):
    """Greedy bin-pack nodes into blocks of <=128 rows, balancing edge load.
    Returns (blk, rowin) arrays."""
    import heapq
    n = len(deg)
    order = np.argsort(-deg, kind="stable")
    heap = [(0, b0 + i, 0) for i in range(nblocks)]
    heapq.heapify(heap)
    blk = np.empty(n, np.int32)
    rowin = np.empty(n, np.int32)
    for node in order:
        while True:
            load, b, cnt = heapq.heappop(heap)
            if cnt < 128:
                break
            # block full: drop it from the heap permanently
        blk[node] = b
        rowin[node] = cnt
        heapq.heappush(heap, (load + int(deg[node]), b, cnt + 1))
    return blk, rowin


def host_prep(cfg, user_emb, item_emb, edge_val, edge_row, edge_col):
    """Node->slot permutation (piece-major table layout), block->core
    assignment, per-core edge tile template (uniform across cores) and
    packed device input arrays."""
    N = cfg.N_USERS + cfg.N_ITEMS
    rng = np.random.default_rng(0)
    deg = np.bincount(edge_row, minlength=N)

    ub, ur = _pack_nodes(deg[:cfg.N_USERS], cfg.U_BLOCKS, 0, rng)
    ib, ir = _pack_nodes(deg[cfg.N_USERS:], cfg.I_BLOCKS, cfg.U_BLOCKS, rng)
    blk = np.concatenate([ub, ib])
    rowin = np.concatenate([ur, ir])

    # block -> (core, k); user blocks get k in [0, U_KPC), items [U_KPC, KPC)
    bload = np.bincount(blk[edge_row], minlength=cfg.BLOCKS)
    core_of = np.empty(cfg.BLOCKS, np.int32)
    k_of = np.empty(cfg.BLOCKS, np.int32)
    for lo, hi, k0 in ((0, cfg.U_BLOCKS, 0), (cfg.U_BLOCKS, cfg.BLOCKS, cfg.U_KPC)):
        ids = np.arange(lo, hi)
        order = ids[np.argsort(-bload[lo:hi], kind="stable")]
        cnt = np.zeros(cfg.N_CORES, np.int32)
        for i, b in enumerate(order):
            rnd, pos = divmod(i, cfg.N_CORES)
            c = pos if rnd % 2 == 0 else cfg.N_CORES - 1 - pos
            core_of[b] = c
            k_of[b] = k0 + cnt[c]
            cnt[c] += 1

    # piece-major global slot: piece p holds all cores' k in [k0p, k1p),
    # core-major within the piece, so AllGather piece output == table piece
    pieces = cfg.PIECES
    pk0 = np.empty(cfg.KPC, np.int64)     # piece start k, per k
    pkn = np.empty(cfg.KPC, np.int64)     # piece size in blocks, per k
    pbase = np.empty(cfg.KPC, np.int64)   # piece base slot, per k
    base = 0
    for (k0, k1) in pieces:
        pk0[k0:k1] = k0
        pkn[k0:k1] = k1 - k0
        pbase[k0:k1] = base
        base += cfg.N_CORES * (k1 - k0) * 128

    def slots_of_all_nodes():
        c_, k_ = core_of[blk], k_of[blk]
        return pbase[k_] + (c_ * pkn[k_] + (k_ - pk0[k_])) * 128 + rowin

    # within each core+side, order k slots by chunk-0 edge count to reduce
    # cross-core template padding
    slot_of_node = slots_of_all_nodes()
    chunk_of_node = slot_of_node // cfg.CHUNK
    ccount = np.zeros((cfg.N_CORES, cfg.KPC, cfg.N_CHUNKS), np.int64)
    np.add.at(ccount, (core_of[blk[edge_row]], k_of[blk[edge_row]],
                       chunk_of_node[edge_col]), 1)
    for c in range(cfg.N_CORES):
        for lo, hi in ((0, cfg.U_KPC), (cfg.U_KPC, cfg.KPC)):
            seg = ccount[c, lo:hi]
            key = seg[:, 0] * 1000000 + seg[:, 1]
            perm = np.argsort(key, kind="stable")
            mask = (core_of == c) & (k_of >= lo) & (k_of < hi)
            bids = np.where(mask)[0]
            old_k = k_of[bids] - lo
            inv = np.empty_like(perm)
            inv[perm] = np.arange(len(perm))
            k_of[bids] = lo + inv[old_k]
            ccount[c, lo:hi] = seg[perm]

    slot_of_node = slots_of_all_nodes()
    chunk_of_node = slot_of_node // cfg.CHUNK
    local_of_node = core_of[blk] * cfg.ROWS_PC + k_of[blk] * 128 + rowin

    # recompute actual per-(core,k,chunk) counts after the k reorder
    ccount = np.zeros((cfg.N_CORES, cfg.KPC, cfg.N_CHUNKS), np.int64)
    np.add.at(ccount, (core_of[blk[edge_row]], k_of[blk[edge_row]],
                       chunk_of_node[edge_col]), 1)

    # local-search polish: swap k slots within (core, side) to shrink the
    # padded template sum_{k,c} ceil(max_core/128). NOTE: moving a block to
    # a different k changes its piece, hence its chunk for SOURCE purposes,
    # so only swap within the same piece (keeps chunk_of_node valid).
    bids = np.full((cfg.N_CORES, cfg.KPC), -1, np.int64)
    bids[core_of, k_of] = np.arange(cfg.BLOCKS)
    cc = ccount
    rng2 = np.random.default_rng(1)

    def col_tiles(k):
        return int(np.ceil(cc[:, k, :].max(axis=0) / 128.0).sum())

    # swap candidates: same core, same side, same piece
    cand = []
    for (pk0_, pk1_) in pieces:
        for lo, hi in ((0, cfg.U_KPC), (cfg.U_KPC, cfg.KPC)):
            a, b = max(lo, pk0_), min(hi, pk1_)
            if b - a >= 2:
                cand.append((a, b))
    for it in range(cfg.BAL_ITERS):
        c = it % cfg.N_CORES
        a, b = cand[rng2.integers(len(cand))]
        j1, j2 = rng2.integers(a, b, 2)
        if j1 == j2:
            continue
        before = col_tiles(j1) + col_tiles(j2)
        cc[c, [j1, j2], :] = cc[c, [j2, j1], :]
        if col_tiles(j1) + col_tiles(j2) > before:
            cc[c, [j1, j2], :] = cc[c, [j2, j1], :]
        else:
            b1, b2 = bids[c, j1], bids[c, j2]
            bids[c, j1], bids[c, j2] = b2, b1
            k_of[b1], k_of[b2] = j2, j1

    slot_of_node = slots_of_all_nodes()
    chunk_of_node = slot_of_node // cfg.CHUNK
    local_of_node = core_of[blk] * cfg.ROWS_PC + k_of[blk] * 128 + rowin
    ccount = np.zeros((cfg.N_CORES, cfg.KPC, cfg.N_CHUNKS), np.int64)
    np.add.at(ccount, (core_of[blk[edge_row]], k_of[blk[edge_row]],
                       chunk_of_node[edge_col]), 1)

    # template: tiles per (k, chunk) = ceil(max over cores / 128)
    T = np.ceil(ccount.max(axis=0) / 128).astype(np.int64)  # [KPC, N_CHUNKS]
    tiles_per_block = T.sum(axis=1)                          # [KPC]
    tot_tiles = int(tiles_per_block.sum())
    tot_slots = tot_tiles * 128

    # per-edge fields
    e_blk = blk[edge_row]
    e_core = core_of[e_blk]
    e_k = k_of[e_blk]
    e_rloc = rowin[edge_row]
    e_src = slot_of_node[edge_col]
    e_chunk = e_src // cfg.CHUNK
    e_idx = (e_src % cfg.CHUNK).astype(np.int16)

    # stream order (group, chunk, k): gathers merge the G blocks of a group
    # into one dma_gather per (group, chunk)
    groups = []
    for (k0, k1) in pieces:
        k = k0
        while k < k1:
            groups.append(list(range(k, min(k + cfg.GMERGE, k1))))
            k += cfg.GMERGE
    seg_rank = np.zeros((cfg.KPC, cfg.N_CHUNKS), np.int64)
    seg_tile_off = np.zeros((cfg.KPC, cfg.N_CHUNKS), np.int64)
    acc = 0
    rank = 0
    for grp_ks in groups:
        for c in range(cfg.N_CHUNKS):
            for k in grp_ks:
                seg_rank[k, c] = rank
                rank += 1
                seg_tile_off[k, c] = acc
                acc += T[k, c]

    idx_all = np.zeros((cfg.N_CORES, tot_slots), np.int16)
    rloc_all = np.zeros((cfg.N_CORES, tot_slots), np.float32)
    val_all = np.zeros((cfg.N_CORES, tot_slots), np.float32)
    e_rank = seg_rank[e_k, e_chunk]
    order = np.lexsort((e_rank, e_core))
    eo_core = e_core[order]
    eo_rank = e_rank[order]
    grp = eo_core.astype(np.int64) * (cfg.KPC * cfg.N_CHUNKS) + eo_rank
    uniq, first = np.unique(grp, return_index=True)
    within = np.arange(len(grp)) - np.repeat(first, np.diff(np.append(first, len(grp))))
    pos = seg_tile_off[e_k[order], e_chunk[order]] * 128 + within
    idx_all[eo_core, pos] = e_idx[order]
    rloc_all[eo_core, pos] = e_rloc[order].astype(np.float32)
    val_all[eo_core, pos] = edge_val[order]

    # device layouts
    idx_lay = np.zeros((cfg.N_CORES, 128, tot_slots // 16), np.int16)
    wrap = idx_all.reshape(cfg.N_CORES, -1, 16)
    for rep in range(8):
        idx_lay[:, rep * 16:(rep + 1) * 16, :] = wrap.transpose(0, 2, 1)

    # S^T tiles: one nonzero per partition (edge slot) at column rloc
    # (dest row within block).
    if cfg.S_ONFLY:
        # device builds S = (iota == rloc) * val on DVE; ship only the
        # per-(partition, tile) rloc and val planes (fp16)
        rv_lay = np.stack([
            rloc_all.reshape(cfg.N_CORES, tot_tiles, 128).transpose(0, 2, 1),
            val_all.reshape(cfg.N_CORES, tot_tiles, 128).transpose(0, 2, 1),
        ], axis=2)  # [cores, 128, 2, tot_tiles]
        st_lay = np.ascontiguousarray(
            rv_lay.reshape(cfg.N_CORES, 128, 2 * tot_tiles).astype(np.float16))
    else:
        SC = cfg.SCOLS
        v_hi = val_all.astype(np.float16)
        st_lay = np.zeros((cfg.N_CORES, 128, tot_tiles * SC), np.float16)
        cidx = np.repeat(np.arange(cfg.N_CORES), tot_slots)
        pp_ = np.tile(np.arange(tot_slots) % 128, cfg.N_CORES)
        tt_ = np.tile(np.arange(tot_slots) // 128, cfg.N_CORES)
        rl_ = rloc_all.reshape(-1).astype(np.int64)
        st_lay[cidx, pp_, tt_ * SC + rl_] = v_hi.reshape(-1)
        if cfg.S_HILO:
            v_lo = (val_all - v_hi.astype(np.float32)).astype(np.float16)
            st_lay[cidx, pp_, tt_ * SC + 128 + rl_] = v_lo.reshape(-1)

    # packed embeddings per core (local k-major layout)
    emb_all = np.zeros((cfg.N_CORES * cfg.ROWS_PC, cfg.CON_DIM), np.float32)
    emb_all[local_of_node[:cfg.N_USERS]] = user_emb
    emb_all[local_of_node[cfg.N_USERS:]] = item_emb
    emb_pc = emb_all.reshape(cfg.N_CORES, cfg.ROWS_PC, cfg.CON_DIM)

    return dict(
        local_of_node=local_of_node, T=T, tiles_per_block=tiles_per_block,
        seg_tile_off=seg_tile_off, tot_tiles=tot_tiles, tot_slots=tot_slots,
        idx_lay=idx_lay, st_lay=st_lay, emb_pc=emb_pc, groups=groups,
    )


# ------------------------------------------------------------ bass build
def build_program(cfg, prep):
    import concourse.bacc as bacc
    import concourse.mybir as mybir
    import concourse.tile as tile

    dt = mybir.dt
    F = cfg.CON_DIM
    KPC = cfg.KPC
    T = prep["T"]
    seg_off = prep["seg_tile_off"]
    tiles_pb = prep["tiles_per_block"]
    tot_tiles = prep["tot_tiles"]
    tot_slots = prep["tot_slots"]
    NCH = cfg.N_CHUNKS
    SC = cfg.SCOLS
    pieces = cfg.PIECES
    groups = prep["groups"]
    AFT = mybir.ActivationFunctionType
    ALU = mybir.AluOpType

    nc = bacc.Bacc("TRN2", target_bir_lowering=False, debug=False,
                   num_devices=cfg.N_CORES, num_swdge_queues=4)

    emb_d = nc.dram_tensor("emb", [cfg.ROWS_PC, F], dt.float32, kind="ExternalInput")
    idx_d = nc.dram_tensor("idx", [128, tot_slots // 16], dt.int16, kind="ExternalInput")
    MAXT = max(int(sum(tiles_pb[k] for k in grp_ks)) for grp_ks in groups)
    if cfg.S_ONFLY:
        st_d = nc.dram_tensor("st", [128, 2 * tot_tiles], dt.float16,
                              kind="ExternalInput")
        iotab_d = nc.dram_tensor("iotab", [128, MAXT * 128], dt.float16,
                                 kind="ExternalInput")
    else:
        st_d = nc.dram_tensor("st", [128, tot_tiles * SC], dt.float16,
                              kind="ExternalInput")
    wt_d = nc.dram_tensor("wt", [F, F], dt.float32, kind="ExternalInput")    # W^T
    v0_d = nc.dram_tensor("v0", [128, 2 * 2 * cfg.RP_AVG], dt.float32, kind="ExternalInput")
    ident_d = nc.dram_tensor("ident", [128, 128], dt.float32, kind="ExternalInput")
    ones_d = nc.dram_tensor("ones1", [1, 128], dt.float32, kind="ExternalInput")
    out_d = nc.dram_tensor("out", [cfg.ROWS_PC, 3 * F], dt.int8, kind="ExternalOutput")
    xdbg = None
    if cfg.DEBUG_X:
        xdbg = [nc.dram_tensor(f"xdbg{i}", [cfg.ROWS_PC, F], dt.float32,
                               kind="ExternalOutput") for i in range(3)]

    # internal DRAM: fp16 x tables, piece-major. One tab tensor per gather
    # CHUNK; each AllGather piece writes its row-slice of its chunk's tab.
    piece_rows = [cfg.N_CORES * (k1 - k0) * 128 for (k0, k1) in pieces]
    piece_base = np.concatenate([[0], np.cumsum(piece_rows)])

    def piece_tensors(nm):
        ag, tab = [], []
        for p, (k0, k1) in enumerate(pieces):
            ag.append(nc.dram_tensor(f"{nm}_ag{p}", [(k1 - k0) * 128, F],
                                     dt.float16))
        for c in range(NCH):
            rows = min(cfg.N_SLOTS, (c + 1) * cfg.CHUNK) - c * cfg.CHUNK
            tab.append(nc.dram_tensor(f"{nm}_tab{c}", [rows, F], dt.float16,
                                      addr_space="Shared"))
        return ag, tab

    ag0, tab0 = piece_tensors("t0")
    ag1, tab1 = piece_tensors("t1")
    gr_in = nc.dram_tensor("gr_in", [4 * 128, F], dt.float32)
    gr_out = nc.dram_tensor("gr_out", [4 * 128, F], dt.float32)

    rg = [list(range(cfg.N_CORES))]
    piece_of = {}
    for q, (k0q, k1q) in enumerate(pieces):
        for kk in range(k0q, k1q):
            piece_of[kk] = q
    # piece p's inputs complete at block k1-1; fire 3 blocks later (capped)
    fire_at = {min(k1 - 1 + 3, cfg.KPC - 1): q
               for q, (k0, k1) in enumerate(pieces)}

    def fire_piece(ag, tab, p):
        r0 = int(piece_base[p])
        c = r0 // cfg.CHUNK
        off = r0 - c * cfg.CHUNK
        nc.gpsimd.collective_compute(
            "AllGather", ALU.bypass, replica_groups=rg,
            ins=[ag[p].ap().opt()],
            outs=[tab[c][off:off + piece_rows[p], :].opt()])

    with tile.TileContext(nc) as tc:
        with tc.tile_pool(name="const", bufs=1) as cpool:
            # preload the gpsimd library holding DMAGatherAnt so the ~60us
            # reload overlaps the Gram phase instead of stalling layer 1
            from concourse import library_config
            nc.gpsimd.load_library(library_config.mlp)
            ident_sb = cpool.tile([128, 128], dt.float32, tag="ident")
            nc.sync.dma_start(ident_sb[:], ident_d[:])
            ones_sb = cpool.tile([1, 128], dt.float32, tag="ones1")
            nc.sync.dma_start(ones_sb[:], ones_d[:])
            wt_sb = cpool.tile([128, 2, F], dt.float32, tag="wt")
            nc.sync.dma_start(wt_sb[:, 0, :], wt_d[0:128, :])
            nc.sync.dma_start(wt_sb[:, 1, :], wt_d[128:256, :])
            v0_sb = cpool.tile([128, 2, 2, cfg.RP_AVG], dt.float32, tag="v0")
            nc.sync.dma_start(v0_sb[:], v0_d[:])
            idx_sb = cpool.tile([128, tot_slots // 16], dt.int16, tag="idx")
            nc.sync.dma_start(idx_sb[:], idx_d[:])
            if cfg.S_ONFLY:
                rv_sb = cpool.tile([128, 2, tot_tiles], dt.float16, tag="rv")
                nc.sync.dma_start(rv_sb[:], st_d[:])
                iotab_sb = cpool.tile([128, MAXT, 128], dt.float16, tag="iotab")
                nc.sync.dma_start(iotab_sb[:], iotab_d[:])

            # =================== RP phase a: Gram matrices ===================
            # emb is DMAed once into a persistent stash; phase d reads it
            # from SBUF (stash pool closed before the layers)
            stash_cm = tc.tile_pool(name="stash", bufs=1)
            stpool = stash_cm.__enter__()
            xst = stpool.tile([128, KPC, F], dt.float32, tag="xst")
            with tc.tile_pool(name="rpa_ps", bufs=1, space="PSUM") as gpsum:
                pg = [gpsum.tile([128, F], dt.float32, tag=f"g{i}", name=f"pg{i}") for i in range(4)]
                # pg[0..1] = Gram_u chunks, pg[2..3] = Gram_i chunks
                # symmetric Gram: row-chunk 0 computes [G00|G01] fully; for
                # chunk 1 only G11 (G10 = G01^T, rebuilt after the AllReduce)
                for k in range(KPC):
                    nc.sync.dma_start(xst[:, k, :], emb_d[k * 128:(k + 1) * 128, :])
                    side = 0 if k < cfg.U_KPC else 1
                    first = k == 0 or k == cfg.U_KPC
                    last = k == cfg.U_KPC - 1 or k == KPC - 1
                    nc.tensor.matmul(pg[2 * side][:],
                                     xst[:, k, 0:128],
                                     xst[:, k, :], start=first, stop=last)
                    nc.tensor.matmul(pg[2 * side + 1][:, 128:256],
                                     xst[:, k, 128:256],
                                     xst[:, k, 128:256], start=first, stop=last)
                g_sb = cpool.tile([128, 4, F], dt.float32, tag="gsb")
                for i in range(4):
                    if i % 2 == 0:
                        nc.scalar.activation(g_sb[:, i, :], pg[i][:], AFT.Copy)
                    else:
                        nc.scalar.activation(g_sb[:, i, 128:256],
                                             pg[i][:, 128:256], AFT.Copy)
                        nc.vector.memset(g_sb[:, i, 0:128], 0)
                for i in range(4):
                    nc.sync.dma_start(gr_in[i * 128:(i + 1) * 128, :], g_sb[:, i, :])
            nc.gpsimd.collective_compute(
                "AllReduce", ALU.add, replica_groups=rg,
                ins=[gr_in.ap().opt()], outs=[gr_out.ap().opt()])

            # ====== RP phase b/c: v, coef, W0 = W^T - (c v)(W v)^T per side ======
            vrow2 = cpool.tile([1, 2, F], dt.float32, tag="vrow2")      # v^T per side
            vb2 = cpool.tile([128, 2, F], dt.float32, tag="vb2")        # bcast v^T
            ncoef2 = cpool.tile([128, 2, 1], dt.float32, tag="ncoef2")  # -eta/vTv
            w0_sb = cpool.tile([128, 2, 2, F], dt.float32, tag="w0sb")
            v2_sb = cpool.tile([128, 2, 2, 1], dt.float32, tag="v2")    # [.,side,jc,.]
            with tc.tile_pool(name="rpb", bufs=1) as vpool, \
                 tc.tile_pool(name="rpb_ps", bufs=1, space="PSUM") as vpsum:
                gg = vpool.tile([128, 4, F], dt.float32, tag="gg")
                for i in range(4):
                    nc.sync.dma_start(gg[:, i, :], gr_out[i * 128:(i + 1) * 128, :])
                # rebuild G10 = G01^T in the row-chunk-1 slots
                for side in range(2):
                    ptg = vpsum.tile([128, 128], dt.float32, tag="pvb")
                    nc.tensor.transpose(ptg[:], gg[:, 2 * side, 128:256],
                                        ident_sb[:])
                    nc.scalar.activation(gg[:, 2 * side + 1, 0:128], ptg[:],
                                         AFT.Copy)
                for side in range(2):
                    cur = None
                    for it in range(cfg.RP_ITER):
                        pv = [vpsum.tile([128, cfg.RP_AVG], dt.float32, tag=f"pv{ic}",
                                        name=f"pv{ic}") for ic in range(2)]
                        for ic in range(2):
                            for kc in range(2):
                                rhs_ap = (v0_sb[:, side, kc, :] if cur is None
                                          else cur[:, kc, :])
                                nc.tensor.matmul(
                                    pv[ic][:],
                                    gg[:, 2 * side + kc, ic * 128:(ic + 1) * 128],
                                    rhs_ap,
                                    start=(kc == 0), stop=(kc == 1))
                        nxt = vpool.tile([128, 2, cfg.RP_AVG], dt.float32, tag=f"vk{it}")
                        for ic in range(2):
                            nc.scalar.activation(nxt[:, ic, :], pv[ic][:], AFT.Copy)
                        cur = nxt
                    # v = mean over restarts
                    for ic in range(2):
                        vsum = vpool.tile([128, 1], dt.float32, tag="vs")
                        nc.vector.tensor_reduce(vsum[:], cur[:, ic, :],
                                                mybir.AxisListType.X, ALU.add)
                        nc.scalar.activation(v2_sb[:, side, ic, :], vsum[:], AFT.Copy,
                                             scale=1.0 / cfg.RP_AVG)
                    # vTv
                    pn = vpsum.tile([1, 1], dt.float32, tag="pn")
                    for ic in range(2):
                        nc.tensor.matmul(pn[:], v2_sb[:, side, ic, :], v2_sb[:, side, ic, :],
                                         start=(ic == 0), stop=(ic == 1))
                    recip = vpool.tile([1, 1], dt.float32, tag="rec")
                    nc.vector.reciprocal(recip[:], pn[:])
                    # broadcast -eta/vTv to all partitions
                    pb = vpsum.tile([128, 1], dt.float32, tag="pb")
                    nc.tensor.matmul(pb[:], ones_sb[:], recip[:], start=True, stop=True)
                    nc.scalar.activation(ncoef2[:, side, :], pb[:], AFT.Copy,
                                         scale=-cfg.RP_ETA)
                    # v row vector [1, 256]
                    for ic in range(2):
                        pt = vpsum.tile([1, 128], dt.float32, tag="ptv")
                        nc.tensor.transpose(pt[:], v2_sb[:, side, ic, :], ident_sb[:])
                        nc.scalar.activation(vrow2[:, side, ic * 128:(ic + 1) * 128],
                                             pt[:], AFT.Copy)
                    # broadcast v^T to all partitions: ones^T @ vrow
                    pvb = vpsum.tile([128, F], dt.float32, tag="pvb")
                    nc.tensor.matmul(pvb[:], ones_sb[:], vrow2[:, side, :],
                                     start=True, stop=True)
                    nc.scalar.activation(vb2[:, side, :], pvb[:], AFT.Copy)
                    # wv^T = v^T W^T  [1, 256], broadcast to all partitions
                    pwv = vpsum.tile([1, F], dt.float32, tag="pwv")
                    for ic in range(2):
                        nc.tensor.matmul(pwv[:], v2_sb[:, side, ic, :], wt_sb[:, ic, :],
                                         start=(ic == 0), stop=(ic == 1))
                    wvrow = vpool.tile([1, F], dt.float32, tag="wvrow")
                    nc.scalar.activation(wvrow[:], pwv[:], AFT.Copy)
                    pwb = vpsum.tile([128, F], dt.float32, tag="pwb")
                    nc.tensor.matmul(pwb[:], ones_sb[:], wvrow[:],
                                     start=True, stop=True)
                    wvb = vpool.tile([128, F], dt.float32, tag="wvb")
                    nc.scalar.activation(wvb[:], pwb[:], AFT.Copy)
                    # W0[jc] = wt[jc] + (ncoef*v)[jc-part] * wv^T
                    for jc in range(2):
                        cv = vpool.tile([128, 1], dt.float32, tag="cv")
                        nc.vector.tensor_tensor(cv[:], v2_sb[:, side, jc, :],
                                                ncoef2[:, side, :], ALU.mult)
                        sc = vpool.tile([128, F], dt.float32, tag="sc")
                        nc.vector.tensor_scalar(sc[:], wvb[:], cv[:], None, ALU.mult)
                        nc.vector.tensor_add(w0_sb[:, side, jc, :], sc[:],
                                             wt_sb[:, jc, :])

            # == RP phase d: X' = X + ncoef (Xv) v^T (DVE), H0 = sign(X@W0) ==
            with tc.tile_pool(name="rpd", bufs=3) as dpool, \
                 tc.tile_pool(name="rpd_xt", bufs=3) as dxt, \
                 tc.tile_pool(name="rpd_pt", bufs=2, space="PSUM") as dpt, \
                 tc.tile_pool(name="rpd_ph", bufs=2, space="PSUM") as dph:
                for k in range(KPC):
                    side = 0 if k < cfg.U_KPC else 1
                    p = piece_of[k]
                    k0 = pieces[p][0]
                    xb = xst[:, k, :]
                    # u = X v (per-partition scalar), cu = ncoef * u
                    scr = dpool.tile([128, F], dt.float32, tag="scr")
                    nc.vector.tensor_mul(scr[:], xb, vb2[:, side, :])
                    u = dpool.tile([128, 1], dt.float32, tag="u")
                    nc.vector.tensor_reduce(u[:], scr[:],
                                            mybir.AxisListType.X, ALU.add)
                    cu = dpool.tile([128, 1], dt.float32, tag="cu")
                    nc.vector.tensor_tensor(cu[:], u[:], ncoef2[:, side, :], ALU.mult)
                    # X' = (vrow * cu) + X
                    scv = dpool.tile([128, F], dt.float32, tag="scv")
                    nc.vector.tensor_scalar(scv[:], vb2[:, side, :], cu[:], None,
                                            ALU.mult)
                    xs = dpool.tile([128, F], dt.float32, tag="xs")
                    nc.vector.tensor_add(xs[:], scv[:], xb)
                    xh = dpool.tile([128, F], dt.float16, tag="xh")
                    nc.scalar.activation(xh[:], xs[:], AFT.Copy)
                    nc.sync.dma_start(ag0[p][(k - k0) * 128:(k - k0 + 1) * 128, :], xh[:])
                    if xdbg is not None:
                        nc.sync.dma_start(xdbg[0][k * 128:(k + 1) * 128, :], xs[:])
                    # H0 = sign(X @ W0) via transposed X
                    xt = dxt.tile([128, 2, 128], dt.float32, tag="xt")
                    for c in range(2):
                        pt = dpt.tile([128, 128], dt.float32, tag="pt")
                        nc.tensor.transpose(pt[:], xst[:, k, c * 128:(c + 1) * 128], ident_sb[:])
                        nc.scalar.activation(xt[:, c, :], pt[:], AFT.Copy)
                    ph = dph.tile([128, F], dt.float32, tag="ph")
                    for jc in range(2):
                        nc.tensor.matmul(ph[:], xt[:, jc, :], w0_sb[:, side, jc, :],
                                         start=(jc == 0), stop=(jc == 1))
                    hb = dpool.tile([128, F], dt.int8, tag="hb")
                    nc.scalar.sign(hb[:], ph[:])
                    nc.sync.dma_start(out_d[k * 128:(k + 1) * 128, 0:F], hb[:])
                    # fire piece q a few blocks after its last input so the
                    # collective's input-wait never stalls dispatch
                    if k in fire_at:
                        fire_piece(ag0, tab0, fire_at[k])
            stash_cm.__exit__(None, None, None)

            # ======================== spmm layers ========================
            for L in (1, 2):
                tabs = tab0 if L == 1 else tab1
                with tc.tile_pool(name=f"l{L}_g", bufs=12) as gpool, \
                     tc.tile_pool(name=f"l{L}_s", bufs=3) as spool, \
                     tc.tile_pool(name=f"l{L}_x", bufs=3) as xpool, \
                     tc.tile_pool(name=f"l{L}_xt", bufs=3) as xtpool, \
                     tc.tile_pool(name=f"l{L}_px", bufs=2, space="PSUM") as pxp, \
                     tc.tile_pool(name=f"l{L}_pt", bufs=2, space="PSUM") as ptp, \
                     tc.tile_pool(name=f"l{L}_ph", bufs=2, space="PSUM") as php:
                    W = min(cfg.PREFETCH_W, len(groups))
                    gtiles = {}

                    def emit_gathers(gi, chunks):
                        grp_ks = groups[gi]
                        base = int(seg_off[grp_ks[0], 0])
                        if gi not in gtiles:
                            ntg = int(sum(tiles_pb[k] for k in grp_ks))
                            gtiles[gi] = gpool.tile([128, ntg, F], dt.float16,
                                                    tag="g", name=f"g{L}_{gi}")
                        g = gtiles[gi]
                        for c in chunks:
                            tgc = int(sum(T[k, c] for k in grp_ks))
                            if tgc == 0:
                                continue
                            goff = int(seg_off[grp_ks[0], c])
                            t0 = goff - base
                            tab_ap = tabs[c][:]
                            nc.gpsimd.dma_gather(
                                g[:, t0:t0 + tgc, :], tab_ap,
                                idx_sb[:, goff * 8:(goff + tgc) * 8],
                                tgc * 128, tgc * 128, F,
                                queue_num=(c if c < 2 else
                                           2 + (groups[gi][0] & 2) // 2))

                    # warm-up: chunk-0/1 gathers of the first W groups run
                    # while the last AllGather piece is still in flight
                    for gi in range(W):
                        emit_gathers(gi, range(NCH - 1))
                    for gi, grp_ks in enumerate(groups):
                        kg0 = grp_ks[0]
                        p = piece_of[kg0]
                        k0p = pieces[p][0]
                        base = int(seg_off[kg0, 0])       # first tile of group
                        ntg = int(sum(tiles_pb[k] for k in grp_ks))
                        if gi < W:
                            emit_gathers(gi, (NCH - 1,))
                        else:
                            emit_gathers(gi, range(NCH))
                        g = gtiles.pop(gi)
                        if cfg.S_ONFLY:
                            cmp = spool.tile([128, ntg, 128], dt.float16, tag="cmp")
                            nc.vector.tensor_tensor(
                                cmp[:], iotab_sb[:, 0:ntg, :],
                                rv_sb[:, 0, base:base + ntg].to_broadcast(
                                    [128, ntg, 128]),
                                ALU.is_equal)
                            s_blk = spool.tile([128, ntg, 128], dt.float16, tag="st")
                            nc.vector.tensor_tensor(
                                s_blk[:], cmp[:],
                                rv_sb[:, 1, base:base + ntg].to_broadcast(
                                    [128, ntg, 128]),
                                ALU.mult)
                        else:
                            s_blk = spool.tile([128, ntg, SC], dt.float16, tag="st")
                            nc.sync.dma_start(
                                s_blk[:], st_d[:, base * SC:(base + ntg) * SC])
                        for k in grp_ks:
                            px = pxp.tile([128, F], dt.float32, tag="px")
                            tsegs = [(int(seg_off[k, c]) - base, int(T[k, c]))
                                     for c in range(NCH)]
                            nseq = sum(n for _, n in tsegs)
                            cnt = 0
                            for t0k, ntc in tsegs:
                                for t in range(t0k, t0k + ntc):
                                    nc.tensor.matmul(
                                        px[:], s_blk[:, t, 0:128],
                                        g[:, t, :],
                                        start=(cnt == 0), stop=(cnt == nseq - 1))
                                    cnt += 1
                            x_sb = xpool.tile([128, F], dt.float32, tag="x")
                            nc.scalar.activation(x_sb[:], px[:], AFT.Copy)
                            if L == 1:
                                xh = xpool.tile([128, F], dt.float16, tag="xh")
                                nc.scalar.activation(xh[:], px[:], AFT.Copy)
                                nc.sync.dma_start(
                                    ag1[p][(k - k0p) * 128:(k - k0p + 1) * 128, :],
                                    xh[:])
                            if xdbg is not None:
                                nc.sync.dma_start(xdbg[L][k * 128:(k + 1) * 128, :],
                                                  x_sb[:])
                            xt = xtpool.tile([128, 2, 128], dt.float32, tag="xt")
                            for c in range(2):
                                pt = ptp.tile([128, 128], dt.float32, tag="pt")
                                nc.tensor.transpose(pt[:],
                                                    x_sb[:, c * 128:(c + 1) * 128],
                                                    ident_sb[:])
                                nc.scalar.activation(xt[:, c, :], pt[:], AFT.Copy)
                            ph = php.tile([128, F], dt.float32, tag="ph")
                            for jc in range(2):
                                nc.tensor.matmul(ph[:], xt[:, jc, :], wt_sb[:, jc, :],
                                                 start=(jc == 0), stop=(jc == 1))
                            hb = xpool.tile([128, F], dt.int8, tag="hb")
                            nc.scalar.sign(hb[:], ph[:])
                            nc.sync.dma_start(
                                out_d[k * 128:(k + 1) * 128, L * F:(L + 1) * F],
                                hb[:])
                        if L == 1 and grp_ks[-1] in fire_at:
                            fire_piece(ag1, tab1, fire_at[grp_ks[-1]])
    nc.compile()
    return nc


# --------------------------------------------------------------- runner
def _run(cfg, user_emb, item_emb, hash_W, rp_v0_user, rp_v0_item,
         edge_val, edge_row, edge_col, trace=False):
    prep = host_prep(cfg, user_emb, item_emb, edge_val, edge_row, edge_col)
    nc = build_program(cfg, prep)

    F = cfg.CON_DIM
    wt_np = np.ascontiguousarray(hash_W.T)
    v0_np = np.zeros((128, 2, 2, cfg.RP_AVG), np.float32)
    for side, v0 in ((0, rp_v0_user), (1, rp_v0_item)):
        v0_np[:, side, 0, :] = v0[0:128, :]
        v0_np[:, side, 1, :] = v0[128:256, :]
    v0_np = v0_np.reshape(128, -1)
    ident_np = np.eye(128, dtype=np.float32)
    ones_np = np.ones((1, 128), np.float32)

    maxt = max(int(sum(prep["tiles_per_block"][k] for k in grp_ks))
               for grp_ks in prep["groups"])
    iotab_np = np.ascontiguousarray(
        np.tile(np.arange(128, dtype=np.float16), (128, maxt)))
    in_maps = []
    for c in range(cfg.N_CORES):
        m = {
            "emb": np.ascontiguousarray(prep["emb_pc"][c]),
            "idx": np.ascontiguousarray(prep["idx_lay"][c]),
            "st": np.ascontiguousarray(prep["st_lay"][c]),
            "wt": wt_np, "v0": v0_np, "ident": ident_np, "ones1": ones_np,
        }
        if cfg.S_ONFLY:
            m["iotab"] = iotab_np
        in_maps.append(m)

    from concourse.bass_utils import run_bass_kernel_spmd
    res = run_bass_kernel_spmd(nc, in_maps, core_ids=list(range(cfg.N_CORES)),
                               trace=trace)

    full = np.concatenate([res.results[c]["out"] for c in range(cfg.N_CORES)],
                          axis=0)
    out = full[prep["local_of_node"]]
    return out, res


def kernel(user_emb, item_emb, hash_W, rp_v0_user, rp_v0_item,
           edge_val, edge_row, edge_col):
    cfg = Cfg()
    out, _ = _run(cfg, user_emb, item_emb, hash_W, rp_v0_user, rp_v0_item,
                  edge_val, edge_row, edge_col)
    return out.astype(np.float32)
